# revision 39
# baseline (speedup 1.0000x reference)
"""Trainium2 Bass kernel for nn_Aligner (cross-attention aligner).

Math (per batch element i):
    ex      = ix[i] @ W.T + b          # [L, D]
    eother  = iother[i] @ W.T + b      # [L, D]
    align   = softmax(ex @ eother.T)   # [L, L], softmax over last dim
    out[i]  = align @ iother[i]        # [L, D]

Shapes: B=8, L=2048, D=1024, fp32.  Sharding: batch-parallel, one batch
element per NeuronCore (8 cores), W/b replicated.  No collectives.

All matmuls run in float32r (full PE rate at N>=256).  TRN2 fp32r
rounds matmul inputs to 11 mantissa bits (RNE, HW-verified); engine
writes into f32r tiles round the same way.  An 11-bit logit pipeline is
~3e-2 off the fp32 reference, so precision is recovered via hi/lo
splits: x is stored as xh = rne11(x) (read exactly by the PE) plus
xl = rne11(x - xh), and X@Y = Xh@Yh (f32r) + [Xh@Yl + Xl@Yh].  The
bracketed cross terms (2^-12-scale corrections) are computed in ONE
fp8e4m3 perf_mode=DoubleRow pass at 0.5 cyc/row, interleaving
(Xh*2^-2, Xl*2^10) / (Yl*2^10, Yh*2^-2) pairs so both products carry a
2^8 scale, removed when merging the cross PSUM into the fp32 logits.

For b == 0 (this problem's inputs), align = ix @ G @ iother^T with
G = W^T@W (equal up to a softmax-invariant per-row shift), which
replaces the eother projection with a cheaper symmetric G compute:
G = Wh^T@Wh + C + C^T with C = Wh^T@Wl done once in bf16.  A generic
3-pass fallback program handles b != 0.

Measured: 729,161 ns/core (cost model; PE 77% busy), hardware
max-scale-relative error 3.9e-4 across all 8 batches.

Per-core dataflow:
  phase A: G (or WT hi/lo) in SBUF; ixG-projection + iother-transpose
           blocks interleaved, hi/lo pairs -> per-block DRAM scratch
  phase B, per l-block of 512 rows:
     S = f32r main + fp8-DoubleRow cross align into fp32 E tiles;
     softmax row-max split in halves (first half hidden under align),
     one exp with fused accum_out row-sum; E PE-transposed -> ET;
     out = ET.T @ iother tiles (dual-queue fed), scaled by 1/Z
"""

import numpy as np

import concourse.bass as bass
import concourse.mybir as mybir
import concourse.tile as tile
from concourse import bacc

P = 128          # partitions
L = 2048         # sequence length
D = 1024         # feature dim
NB = 8           # batch / cores
KC = D // P      # 8 contraction chunks for stage-1 matmuls
DG = D // P      # 8 output d-groups of stage 1
NLB = L // 512   # 4 l-blocks of 512
LS = 4           # l-subs of 128 per l-block
MC = L // 512    # 4 m-chunks of 512 for align
M16 = L // P     # 16 m-chunks of 128 for stage 4

F32 = mybir.dt.float32
F32R = mybir.dt.float32r
FP8 = mybir.dt.float8e4
BF16 = mybir.dt.bfloat16

# Include the C = Wh^T@Wl correction in G.  Dropping it costs ~4e-3
# max-scale output error (numpy-modelled; tolerance 2e-2) and saves the
# whole C pipeline in phase A.  Flip to True if hardware error surprises.
USE_C = False
DROW = mybir.MatmulPerfMode.DoubleRow
COPYF = mybir.ActivationFunctionType.Copy
AX = mybir.AxisListType.X
EXP = mybir.ActivationFunctionType.Exp


def build_program(zero_bias=False):
    """zero_bias=True uses the G = W^T@W factorization:
    ex@eother^T = ix@G@iother^T (+ bias terms that vanish for b=0, up to a
    softmax-invariant per-row shift).  This removes the eother projection
    and all W transposes; G costs half an eother projection."""
    nc = bacc.Bacc("TRN2", target_bir_lowering=False, debug=False)

    ix = nc.dram_tensor("ix", [L, D], F32, kind="ExternalInput").ap()
    iother = nc.dram_tensor("iother", [L, D], F32, kind="ExternalInput").ap()
    W = nc.dram_tensor("W", [D, D], F32, kind="ExternalInput").ap()
    bvec = nc.dram_tensor("b", [D], F32, kind="ExternalInput").ap()
    out = nc.dram_tensor("out", [L, D], F32, kind="ExternalOutput").ap()
    # identity for PE transpose-mode, fed from host (avoids f32r memset)
    ident_in = nc.dram_tensor("ident", [P, P], F32R, kind="ExternalInput").ap()

    # staging: projected-transposed activations (hi/lo), phase A -> phase B.
    # One DRAM tensor per 512-block so Tile's per-tensor DRAM dependency
    # tracking lets phase-B reads start as soon as their block is written.
    def scratch(name):
        t = nc.dram_tensor(name, [D, 512], F32R).ap()
        return t.rearrange("(dg p) l -> p dg l", p=P)           # [128, 8, 512]

    exT_h = [scratch(f"exTh_scratch{i}") for i in range(NLB)]
    exT_l = [scratch(f"exTl_scratch{i}") for i in range(NLB)]
    eoT_h = [scratch(f"eoTh_scratch{i}") for i in range(NLB)]
    # eoT lo exists only as the fp8 word inside the SBUF-resident eoq8 in
    # the zero-bias program; the generic program still stages it in DRAM.
    eoT_l = [scratch(f"eoTl_scratch{i}") for i in range(NLB)]

    with tile.TileContext(nc) as tc:
        with (
            tc.tile_pool(name="const", bufs=1) as const,
            tc.tile_pool(name="exTb", bufs=1) as exTb_pool,
            tc.tile_pool(name="eoTb", bufs=2) as eoTb_pool,
            tc.tile_pool(name="psum_tp", bufs=2, space="PSUM") as psum_tp,
            tc.tile_pool(name="psum_mm", bufs=2, space="PSUM") as psum_mm,
            tc.tile_pool(name="psum_s4", bufs=4, space="PSUM") as psum_s4,
        ):
            identr = const.tile([P, P], F32R, name="identr")
            nc.gpsimd.dma_start(out=identr, in_=ident_in)
            identf = identr.bitcast(F32)
            identb = const.tile([P, P], BF16, name="identb")
            nc.scalar.copy(out=identb, in_=identr)

            if zero_bias:
                # SBUF-resident fp8 (Yl*2^13, Yh*2^4) pairs of the FULL
                # eoT, built once during the phase-A transposes.  Phase B's
                # align cross pass reads slices directly -- no per-l-block
                # eoT-lo streaming and no per-chunk yq rebuilds.
                eoq8_sb = const.tile([P, DG, 2, L], FP8, name="eoq8_sb")

            # b laid out [p, dg]: btile[p, dg] = b[dg*128 + p]
            # (only the generic-bias path reads it)
            if not zero_bias:
                btile = const.tile([P, DG], F32)
                nc.sync.dma_start(out=btile,
                                  in_=bvec.rearrange("(c p) -> p c", p=P))

            def transpose_128_group(src_row, dst, rdtype=False, bf=False):
                """Transpose four [128,128] slices through one PSUM bank;
                single eviction into dst ([128, 4, 128] SBUF AP).
                rdtype=False: fp32 transpose-mode (bit-exact, 2 cyc/row);
                rdtype=True: f32r mode (rounds to 11 bits, 1.5 cyc/row);
                bf=True: bf16 in/out (1 cyc/row)."""
                dt = BF16 if bf else (F32R if rdtype else F32)
                idt = identb if bf else (identr if rdtype else identf)
                tp = psum_tp.tile([P, 4 * P], dt, tag="tp", name="tpg")
                for i in range(4):
                    nc.tensor.transpose(
                        tp[:, i * P:(i + 1) * P],
                        src_row[:, i * P:(i + 1) * P],
                        idt,
                    )
                nc.scalar.copy(out=dst, in_=tp.rearrange(
                    "p (four c) -> p four c", four=4))

            _tp_rr = [0]

            def transpose_128_group_hl(src_row, dst_h, dst_l, borrow=False):
                """Like transpose_128_group, but evicts an f32r hi/lo pair:
                hi = rne11(psum) via ACT/DVE copy (alternating), lo = psum -
                hi via DVE sub.  borrow=True also rotates through the
                (phase-A-idle) stage-4 PSUM banks for a deeper transpose
                pipeline."""
                _tp_rr[0] += 1
                if borrow and _tp_rr[0] % 3 != 0:
                    tp = psum_s4.tile([P, 4 * P], F32, tag="s4",
                                      name=f"tpb{_tp_rr[0]}")
                else:
                    tp = psum_tp.tile([P, 4 * P], F32, tag="tp",
                                      name=f"tpt{_tp_rr[0]}")
                for i in range(4):
                    nc.tensor.transpose(
                        tp[:, i * P:(i + 1) * P],
                        src_row[:, i * P:(i + 1) * P],
                        identf,
                    )
                tp4 = tp.rearrange("p (four c) -> p four c", four=4)
                if _tp_rr[0] % 2 == 0:
                    nc.scalar.copy(out=dst_h, in_=tp4)
                else:
                    nc.vector.tensor_copy(out=dst_h, in_=tp4)
                nc.vector.tensor_sub(out=dst_l, in0=tp4, in1=dst_h)

            # ---------------- phase A: WTh/WTl + exT/eoT (hi/lo) -> DRAM ----
            with (
                tc.tile_pool(name="wt", bufs=1) as wt_pool,
                tc.tile_pool(name="stage", bufs=3) as stage_pool,
                tc.tile_pool(name="ev", bufs=1) as ev_pool,
                tc.tile_pool(name="evt", bufs=2) as evt_pool,
            ):
                # lhsT hi/lo pair for the ix projection:
                #   direct path: WT (transposed W);  G path: G = W^T@W
                #   (symmetric, so its [i-part, j] layout is its own lhsT)
                wth = wt_pool.tile([P, KC, D], F32R)
                if zero_bias:
                    # G-lo lives only as the fp8 word in wq8 (slot 1); no
                    # f32r wtl tile is kept in this mode.
                    wq8 = wt_pool.tile([P, KC, 2, D], FP8, name="wq8")
                else:
                    wtl = wt_pool.tile([P, KC, D], F32R)

                if zero_bias:
                  # G = W^T@W via hi/lo.  G is symmetric, so the two cross
                  # terms are each other's transposes: Wh^T@Wl = (Wl^T@Wh)^T.
                  # C = Wh^T@Wl is a 2^-12-scale correction, so it runs in
                  # pure bf16 (err ~2^-21 * G).  C^T is also lo-scale, so it
                  # is added into the LO part after the hi/lo split, reading
                  # the transpose PSUM directly -- no C^T SBUF tensor.
                  with (
                      tc.tile_pool(name="split", bufs=2) as split_pool,
                      tc.tile_pool(name="cpool", bufs=1) as c_pool,
                  ):
                    if USE_C:
                        cmat = c_pool.tile([P, KC, D], BF16, name="cmat")
                        # fp8 caches of Wh*2^4 / Wl*2^16, laid out as
                        # DoubleRow pairs over adjacent 128-row contraction
                        # chunks: the C pass runs at 0.5 cyc/row.  C is a
                        # 2^-12-scale correction, so fp8's 2^-4 word
                        # precision gives C to 2^-16 overall.  Product
                        # scale 2^20 is removed at the cmat eviction.
                        whc8 = c_pool.tile([P, KC // 2, 2, D], FP8,
                                           name="whc8")
                        wlc8 = c_pool.tile([P, KC // 2, 2, D], FP8,
                                           name="wlc8")

                    def g_psums(pfx):
                        return ([psum_mm.tile([P, 512], F32, tag="mm",
                                              name=f"{pfx}_{i}")
                                 for i in range(2)]
                                + [psum_s4.tile([P, 512], F32, tag="s4",
                                                name=f"{pfx}_{i + 2}")
                                   for i in range(4)]
                                + [psum_tp.tile([P, 512], F32, tag="tp",
                                                name=f"{pfx}_{i + 6}")
                                   for i in range(2)])

                    # fp8 pair caches of Wh / Wl
                    for dc in range(DG if USE_C else 0):
                        wrow = stage_pool.tile([P, D], F32, tag="stage",
                                               name=f"gw{dc}")
                        nc.sync.dma_start(
                            out=wrow, in_=W[dc * P:(dc + 1) * P, :])
                        whr = split_pool.tile([P, D], F32R, tag="whc",
                                              name=f"gwh{dc}")
                        nc.vector.tensor_copy(out=whr, in_=wrow)
                        nc.scalar.activation(
                            out=whc8[:, dc // 2, dc % 2, :], in_=wrow,
                            func=COPYF, scale=16.0)
                        wl32 = split_pool.tile([P, D], F32, tag="wl32",
                                               name=f"gwl{dc}")
                        nc.vector.tensor_sub(out=wl32, in0=wrow, in1=whr)
                        nc.scalar.activation(
                            out=wlc8[:, dc // 2, dc % 2, :], in_=wl32,
                            func=COPYF, scale=65536.0)

                    # C = Wh^T @ Wl, fp8 DoubleRow (pairs = adjacent chunks)
                    for jh in range(2 if USE_C else 0):
                        jsl = slice(jh * 512, (jh + 1) * 512)
                        pss = g_psums(f"c{jh}")
                        for dp in range(DG // 2):
                            for ic in range(DG):
                                nc.tensor.matmul(
                                    pss[ic],
                                    whc8[:, dp, :, ic * P:(ic + 1) * P],
                                    wlc8[:, dp, :, jsl],
                                    start=(dp == 0), stop=(dp == DG // 2 - 1),
                                    perf_mode=DROW)
                        for ic in range(DG):
                            nc.scalar.activation(
                                out=cmat[:, ic, jsl], in_=pss[ic],
                                func=COPYF, scale=2.0 ** -20)

                    # G = Wh^T@Wh + C + C^T; Wh streamed per (jh, dc)
                    for jh in range(2):
                        jsl = slice(jh * 512, (jh + 1) * 512)
                        pss = g_psums(f"g{jh}")
                        for dc in range(DG):
                            wrow = stage_pool.tile([P, D], F32, tag="stage",
                                                   name=f"g2w{jh}_{dc}")
                            nc.sync.dma_start(
                                out=wrow, in_=W[dc * P:(dc + 1) * P, :])
                            whc = split_pool.tile([P, D], F32R, tag="whc",
                                                  name=f"g2wh{jh}_{dc}")
                            nc.vector.tensor_copy(out=whc, in_=wrow)
                            for ic in range(DG):
                                nc.tensor.matmul(
                                    pss[ic], whc[:, ic * P:(ic + 1) * P],
                                    whc[:, jsl],
                                    start=(dc == 0), stop=(dc == DG - 1))
                        for ic in range(DG):
                            if USE_C:
                                tmp = split_pool.tile(
                                    [P, 512], F32, tag="gtmp",
                                    name=f"ga{jh}_{ic}")
                                nc.vector.tensor_add(out=tmp, in0=pss[ic],
                                                     in1=cmat[:, ic, jsl])
                            else:
                                tmp = pss[ic]
                            nc.scalar.copy(out=wth[:, ic, jsl], in_=tmp)
                            gl = split_pool.tile([P, 512], F32, tag="gl",
                                                 name=f"gl{jh}_{ic}")
                            nc.vector.tensor_sub(out=gl, in0=tmp,
                                                 in1=wth[:, ic, jsl])
                            if USE_C:
                                # C^T via PE transposes of cmat, read from
                                # PSUM.  Allocated from psum_mm (not
                                # psum_tp): the tp slots are held by
                                # pss[6]/pss[7] until the last evictions,
                                # and the gl/gtmp slot chain feeds back
                                # into ctp -- psum_tp here would deadlock.
                                ctp = psum_mm.tile([P, 4 * P], BF16,
                                                   tag="mm",
                                                   name=f"ctp{jh}_{ic}")
                                for t in range(4):
                                    jc = jh * 4 + t
                                    nc.tensor.transpose(
                                        ctp[:, t * P:(t + 1) * P],
                                        cmat[:, jc, ic * P:(ic + 1) * P],
                                        identb)
                                nc.vector.tensor_add(
                                    out=gl, in0=gl,
                                    in1=ctp.rearrange(
                                        "p (four c) -> p four c", four=4))
                            # G-lo straight to its fp8 DoubleRow word
                            # (Gl*2^13); pairs with xh*2^-13 in the proj
                            # cross pass so both slot products land at
                            # scale 2^0 vs the f32r hi*hi pass and the
                            # cross pass accumulates into the SAME psum.
                            nc.scalar.activation(out=wq8[:, ic, 1, jsl],
                                                 in_=gl, func=COPYF,
                                                 scale=8192.0)
                    # hi word: Gh*2^2, pairs with xl*2^-2
                    nc.scalar.activation(out=wq8[:, :, 0, :], in_=wth,
                                         func=COPYF, scale=4.0)
                else:
                    for dc in range(DG):
                        wrow = stage_pool.tile([P, D], F32, tag="stage",
                                               name=f"wrow{dc}")
                        nc.sync.dma_start(out=wrow,
                                          in_=W[dc * P:(dc + 1) * P, :])
                        for q in range(2):
                            transpose_128_group_hl(
                                wrow[:, q * 4 * P:(q + 1) * 4 * P],
                                wth[:, q * 4:(q + 1) * 4, dc * P:(dc + 1) * P],
                                wtl[:, q * 4:(q + 1) * 4, dc * P:(dc + 1) * P],
                                borrow=True)

                with (
                    tc.tile_pool(name="xT", bufs=1) as xT_pool,
                    tc.tile_pool(name="ev", bufs=1) as ev_pool,
                    tc.tile_pool(name="evt", bufs=2) as evt_pool,
                    tc.tile_pool(name="iotp", bufs=2) as iotp_pool,
                ):
                    def tp_block(src_dram, dst_h, dst_l, pfx, blk):
                        """dst = src_blk^T (hi/lo split), via small per-group
                        tiles DMA'd out immediately -- fills PE bubbles between
                        projection blocks without big-slot contention.
                        zero_bias: hi also goes to DRAM (SWDGE queue -- the
                        sync queue is phase A's bottleneck), lo goes ONLY
                        into the resident eoq8 fp8 pairs."""
                        for s in range(4):
                            row = stage_pool.tile([P, D], F32, tag="stage",
                                                  name=f"{pfx}row{blk}_{s}")
                            r0 = (blk * 4 + s) * P
                            nc.sync.dma_start(out=row, in_=src_dram[r0:r0 + P, :])
                            ssl = slice(s * P, (s + 1) * P)
                            for q in range(2):
                                th = iotp_pool.tile([P, 4, P], F32R, tag="ioh",
                                                    name=f"{pfx}h{blk}_{s}_{q}")
                                tl = iotp_pool.tile([P, 4, P], F32R, tag="iol",
                                                    name=f"{pfx}l{blk}_{s}_{q}")
                                transpose_128_group_hl(
                                    row[:, q * 4 * P:(q + 1) * 4 * P], th, tl)
                                qsl = slice(q * 4, (q + 1) * 4)
                                if zero_bias:
                                    nc.gpsimd.dma_start(
                                        out=dst_h[blk][:, qsl, ssl], in_=th)
                                    lr = slice(blk * 512 + s * P,
                                               blk * 512 + (s + 1) * P)
                                    nc.scalar.activation(
                                        out=eoq8_sb[:, qsl, 1, lr], in_=th,
                                        func=COPYF, scale=16.0)
                                    nc.vector.tensor_scalar_mul(
                                        out=eoq8_sb[:, qsl, 0, lr], in0=tl,
                                        scalar1=8192.0)
                                else:
                                    nc.sync.dma_start(
                                        out=dst_h[blk][:, qsl, ssl], in_=th)
                                    nc.sync.dma_start(
                                        out=dst_l[blk][:, qsl, ssl], in_=tl)

                    def proj_block(src_dram, dst_h, dst_l, pfx, blk):
                        """dst[blk] = lhsT_pair @ src_blk^T + b, stored
                        hi/lo.  zero_bias: x is transposed with a 4096*I
                        identity (x carries 2^12), the hi*hi pass runs in
                        f32r and the two cross terms run in ONE fp8
                        DoubleRow pass accumulating into the same psum.
                        Generic path: 3-pass f32r hi/lo.  Processed as two
                        256-halves with double-buffered xh/xl so half h+1's
                        transposes and evictions overlap half h's matmuls."""
                        for hf in range(2):
                            xh = xT_pool.tile([P, KC, 256], F32R, tag="xh",
                                              bufs=2, name=f"{pfx}xh{blk}_{hf}")
                            xl = xT_pool.tile([P, KC, 256], F32R, tag="xl",
                                              bufs=2, name=f"{pfx}xl{blk}_{hf}")
                            for si in range(2):
                                s_ = hf * 2 + si
                                row = stage_pool.tile(
                                    [P, D], F32, tag="stage",
                                    name=f"{pfx}row{blk}_{s_}")
                                r0 = (blk * 4 + s_) * P
                                nc.sync.dma_start(
                                    out=row, in_=src_dram[r0:r0 + P, :])
                                if zero_bias:
                                    # x carries 2^12 through proj and align
                                    # (see wq8 comment); exact 2^k scaling,
                                    # in place before the transpose
                                    nc.scalar.activation(
                                        out=row, in_=row, func=COPYF,
                                        scale=4096.0)
                                ssl = slice(si * P, (si + 1) * P)
                                for q in range(2):
                                    transpose_128_group_hl(
                                        row[:, q * 4 * P:(q + 1) * 4 * P],
                                        xh[:, q * 4:(q + 1) * 4, ssl],
                                        xl[:, q * 4:(q + 1) * 4, ssl],
                                        borrow=True)
                            if zero_bias:
                                xq8 = xT_pool.tile(
                                    [P, KC, 2, 256], FP8, tag="xq8", bufs=2,
                                    name=f"{pfx}xq8{blk}_{hf}")
                                nc.vector.tensor_scalar_mul(
                                    out=xq8[:, :, 0, :], in0=xl, scalar1=0.25)
                                nc.scalar.activation(
                                    out=xq8[:, :, 1, :], in_=xh, func=COPYF,
                                    scale=2.0 ** -13)
                            hsl = slice(hf * 256, (hf + 1) * 256)
                            for dg in range(DG):
                                if dg % 2 == 0:
                                    evh = ev_pool.tile(
                                        [P, 2, 256], F32R, tag="evh", bufs=2,
                                        name=f"{pfx}evh{blk}_{hf}_{dg}")
                                    evl = ev_pool.tile(
                                        [P, 2, 256], F32R, tag="evl", bufs=2,
                                        name=f"{pfx}evl{blk}_{hf}_{dg}")
                                if zero_bias:
                                    # two dg share one [P, 2, 256] psum
                                    # bank so the hi/lo eviction runs as a
                                    # single [P, 512] op pair
                                    if dg % 2 == 0:
                                        ps2 = psum_mm.tile(
                                            [P, 2, 256], F32, tag="mm",
                                            name=f"{pfx}ps{blk}_{hf}_{dg}")
                                    ps = ps2[:, dg % 2, :]
                                    for kc in range(KC):
                                        nc.tensor.matmul(
                                            ps,
                                            wth[:, kc, dg * P:(dg + 1) * P],
                                            xh[:, kc, :],
                                            start=(kc == 0), stop=False,
                                        )
                                    for kc in range(KC):
                                        nc.tensor.matmul(
                                            ps,
                                            wq8[:, kc, :, dg * P:(dg + 1) * P],
                                            xq8[:, kc, :, :],
                                            start=False, stop=(kc == KC - 1),
                                            perf_mode=DROW,
                                        )
                                else:
                                    ps = psum_mm.tile(
                                        [P, 256], F32, tag="mm",
                                        name=f"{pfx}ps{blk}_{hf}_{dg}")
                                    n = 0
                                    for wt_t, x_t in ((wth, xh), (wth, xl),
                                                      (wtl, xh)):
                                        for kc in range(KC):
                                            nc.tensor.matmul(
                                                ps,
                                                wt_t[:, kc, dg * P:(dg + 1) * P],
                                                x_t[:, kc, :],
                                                start=(n == 0),
                                                stop=(n == 3 * KC - 1),
                                            )
                                            n += 1
                                if zero_bias:
                                    if dg % 2 == 1:
                                        nc.scalar.copy(out=evh, in_=ps2)
                                        nc.vector.tensor_sub(
                                            out=evl, in0=ps2, in1=evh)
                                else:
                                    tmp = evt_pool.tile(
                                        [P, 256], F32, tag="evt",
                                        name=f"{pfx}tmp{blk}_{hf}_{dg}")
                                    nc.vector.tensor_scalar_add(
                                        out=tmp, in0=ps,
                                        scalar1=btile[:, dg:dg + 1])
                                    nc.vector.tensor_copy(
                                        out=evh[:, dg % 2, :], in_=tmp)
                                    nc.vector.tensor_sub(
                                        out=evl[:, dg % 2, :], in0=tmp,
                                        in1=evh[:, dg % 2, :])
                                if dg % 2 == 1:
                                    dsl = slice(dg - 1, dg + 1)
                                    nc.sync.dma_start(
                                        out=dst_h[blk][:, dsl, hsl], in_=evh)
                                    # lo on the SWDGE queue: the sync queue
                                    # is phase A's dispatch bottleneck
                                    (nc.gpsimd if zero_bias else nc.sync
                                     ).dma_start(
                                        out=dst_l[blk][:, dsl, hsl], in_=evl)

                    if zero_bias:
                        # ALL io-transpose blocks first: phase B's first
                        # align chunk needs the complete eoq8_sb, and the
                        # transposes also fill the PE while the G evictions
                        # drain on ACT/DVE.  proj blocks follow; phase B's
                        # lb=0 can start right after proj block 0 lands.
                        for blk in range(NLB):
                            tp_block(iother, eoT_h, eoT_l, "eo", blk)
                        for blk in range(NLB):
                            proj_block(ix, exT_h, exT_l, "ex", blk)
                    else:
                        for blk in range(NLB):
                            proj_block(ix, exT_h, exT_l, "ex", blk)
                        for blk in range(NLB):
                            proj_block(iother, eoT_h, eoT_l, "eo", blk)

            # ---------------- phase B: align + softmax + output -------------
            with (
                tc.tile_pool(name="epool", bufs=4) as e_pool,
                tc.tile_pool(name="q8", bufs=1) as q8_pool,
                tc.tile_pool(name="c32", bufs=4) as c32_pool,
                tc.tile_pool(name="etpool", bufs=4) as et_pool,
                tc.tile_pool(name="s4rhs", bufs=4) as s4rhs_pool,
                tc.tile_pool(name="outp", bufs=4) as out_pool,
                tc.tile_pool(name="small", bufs=10) as small_pool,
            ):
                for lb in range(NLB):
                    exbh = exTb_pool.tile([P, DG, 512], F32R, tag="exbh",
                                          name=f"exbh{lb}")
                    exbl = exTb_pool.tile([P, DG, 512], F32R, tag="exbl",
                                          name=f"exbl{lb}")
                    # SWDGE queue: lets these overtake phase-A writes still
                    # pending in the sync-engine HWDGE FIFO
                    nc.gpsimd.dma_start(out=exbh, in_=exT_h[lb])
                    nc.gpsimd.dma_start(out=exbl, in_=exT_l[lb])

                    NMC = 2 * MC      # 8 chunks of 256
                    Es = [e_pool.tile([P, L], F32, tag="E",
                                      name=f"E{lb}_{i}") for i in range(LS)]
                    nms = {}
                    if zero_bias:
                        # fp8 interleaved (Xh*2^-13, Xl*2^-4) pairs of exT
                        # (X carries 2^12), built once per l-block on DVE.
                        # Paired with y words (Yl*2^13, Yh*2^4) both slot
                        # products land at scale 2^0 vs the f32r hi*hi pass,
                        # so the cross pass accumulates into the same psum.
                        exq8 = q8_pool.tile([P, DG, 2, 512], FP8, tag="exq8",
                                            name=f"exq8{lb}")
                        nc.vector.tensor_scalar_mul(
                            out=exq8[:, :, 0, :], in0=exbh, scalar1=2.0 ** -13)
                        nc.scalar.activation(
                            out=exq8[:, :, 1, :], in_=exbl, func=COPYF,
                            scale=2.0 ** -4)
                    for mc in range(NMC):
                        msl = slice(mc * 256, (mc + 1) * 256)
                        blk_i, half = mc // 2, mc % 2
                        hsl = slice(half * 256, (half + 1) * 256)
                        eobh = eoTb_pool.tile([P, DG, 256], F32R, tag="eobh",
                                              name=f"eobh{lb}_{mc}")
                        nc.gpsimd.dma_start(out=eobh,
                                            in_=eoT_h[blk_i][:, :, hsl])
                        if zero_bias:
                            # fp8 (Yl*2^13, Yh*2^4) pairs come straight from
                            # the resident eoq8_sb -- no lo stream, no build
                            yq = eoq8_sb[:, :, :, msl]
                        else:
                            eobl = eoTb_pool.tile([P, DG, 256], F32R,
                                                  tag="eobl",
                                                  name=f"eobl{lb}_{mc}")
                            nc.gpsimd.dma_start(out=eobl,
                                                in_=eoT_l[blk_i][:, :, hsl])
                        for ls in range(LS):
                            xsl = slice(ls * P, (ls + 1) * P)
                            if zero_bias:
                                # hi*hi f32r pass + cross terms Xh@Yl+Xl@Yh
                                # in ONE fp8 DoubleRow continuation of the
                                # SAME psum accumulation (scales match)
                                ps = psum_mm.tile([P, 256], F32, tag="mm",
                                                  name=f"al{lb}_{mc}_{ls}")
                                for dc in range(DG):
                                    nc.tensor.matmul(
                                        ps, exbh[:, dc, xsl], eobh[:, dc, :],
                                        start=(dc == 0), stop=False,
                                    )
                                for dc in range(DG):
                                    nc.tensor.matmul(
                                        ps,
                                        exq8[:, dc, :, xsl],
                                        yq[:, dc, :, :],
                                        start=False,
                                        stop=(dc == DG - 1),
                                        perf_mode=DROW,
                                    )
                                # E is fp32: store raw 2^12-scaled logits;
                                # the PSUM is freed after this single read
                                # (alternate engines to balance load)
                                if (mc + ls) % 2 == 0:
                                    nc.vector.tensor_copy(
                                        out=Es[ls][:, msl], in_=ps)
                                else:
                                    nc.scalar.copy(
                                        out=Es[ls][:, msl], in_=ps)
                                if mc == 3:
                                    # first-half row max, hidden under the
                                    # align of chunks 4-7
                                    nms[ls] = small_pool.tile(
                                        [P, 1], F32, tag="nm1",
                                        name=f"nm1_{lb}_{ls}")
                                    nc.vector.reduce_max(
                                        nms[ls], Es[ls][:, :1024], axis=AX,
                                        negate=True)
                                continue
                            ps = psum_mm.tile([P, 256], F32, tag="mm",
                                              name=f"al{lb}_{mc}_{ls}")
                            n = 0
                            for x_t, eo_t in ((exbh, eobh), (exbh, eobl),
                                              (exbl, eobh)):
                                for dc in range(DG):
                                    nc.tensor.matmul(
                                        ps,
                                        x_t[:, dc, ls * P:(ls + 1) * P],
                                        eo_t[:, dc, :],
                                        start=(n == 0), stop=(n == 3 * DG - 1),
                                    )
                                    n += 1
                            nc.vector.tensor_copy(out=Es[ls][:, msl],
                                                    in_=ps)

                    ets = []
                    rzs = []
                    for ls in range(LS):
                        E = Es[ls]
                        negM = small_pool.tile([P, 1], F32, tag="negM",
                                               name=f"nm{lb}_{ls}")
                        if zero_bias:
                            nc.vector.reduce_max(negM, E[:, 1024:], axis=AX,
                                                 negate=True)
                            # -max(a,b) = min(-a,-b)
                            nc.vector.tensor_tensor(
                                out=negM, in0=negM, in1=nms[ls],
                                op=mybir.AluOpType.min)
                        else:
                            nc.vector.reduce_max(negM, E, axis=AX,
                                                 negate=True)
                        zsum = small_pool.tile([P, 1], F32, tag="zsum",
                                               name=f"zs{lb}_{ls}")
                        if zero_bias:
                            # logits carry 2^12; EXP's scale knob removes it
                            # (bias must then be -max * 2^-12 as well).
                            # exp output goes to an f32r tile: 1.5
                            # cyc/row E-transposes instead of fp32's 2
                            # (bf16 at 1.0 is rejected by the NEFF
                            # compiler when mixed with the f32r stage-4
                            # rhs: "Mixing of 32-bit and non-32-bit
                            # Matmult inputs not supported").
                            negMs = small_pool.tile([P, 1], F32, tag="negMs",
                                                    name=f"nms{lb}_{ls}")
                            nc.vector.tensor_scalar_mul(
                                out=negMs, in0=negM, scalar1=2.0 ** -12)
                            E2 = e_pool.tile([P, L], F32R, tag="E2",
                                             bufs=2, name=f"E2_{lb}_{ls}")
                            nc.scalar.activation(
                                out=E2, in_=E, func=EXP, bias=negMs,
                                scale=2.0 ** -12, accum_out=zsum)
                        else:
                            nc.scalar.activation(
                                out=E, in_=E, func=EXP, bias=negM, scale=1.0,
                                accum_out=zsum)
                        rz = small_pool.tile([P, 1], F32, tag="rz",
                                             name=f"rz{lb}_{ls}")
                        nc.vector.reciprocal(rz, zsum)
                        rzs.append(rz)
                        # ET[p, m16, l] = E[l, m16*128 + p]
                        ET = et_pool.tile([P, M16, P], F32R,
                                          tag="ET", name=f"ET{lb}_{ls}")
                        for q in range(4):
                            transpose_128_group(
                                (E2 if zero_bias else E)[:, q * 4 * P:
                                                         (q + 1) * 4 * P],
                                ET[:, q * 4:(q + 1) * 4, :],
                                rdtype=zero_bias)
                        ets.append(ET)

                    # stage 4: out rows = (E @ iother) * rz.  One [P, 1024]
                    # rhs load per m16 feeds BOTH d-halves (half the DMA
                    # dispatches); 8 psum banks (s4 + borrowed mm/tp, all
                    # idle here) hold the full 512x1024 output block.
                    pss4 = (
                        [psum_s4.tile([P, 512], F32, tag="s4",
                                      name=f"s4_{lb}_{i}") for i in range(4)]
                        + [psum_mm.tile([P, 512], F32, tag="mm",
                                        name=f"s4m_{lb}_{i}") for i in range(2)]
                        + [psum_tp.tile([P, 512], F32, tag="tp",
                                        name=f"s4t_{lb}_{i}") for i in range(2)]
                    )
                    for m16 in range(M16):
                        rhs = s4rhs_pool.tile([P, D], F32R, tag="s4rhs",
                                              name=f"rhs{lb}_{m16}")
                        # feed the wave from BOTH DMA queues: one queue
                        # alone cannot keep up with the PE
                        eng = nc.sync if m16 % 2 == 0 else nc.gpsimd
                        eng.dma_start(
                            out=rhs,
                            in_=iother[m16 * P:(m16 + 1) * P, :].bitcast(F32R))
                        for dg in range(2):
                            for ls in range(LS):
                                nc.tensor.matmul(
                                    pss4[dg * 4 + ls],
                                    ets[ls][:, m16, :],
                                    rhs[:, dg * 512:(dg + 1) * 512],
                                    start=(m16 == 0), stop=(m16 == M16 - 1),
                                )
                    for dg in range(2):
                        for ls in range(LS):
                            ot = out_pool.tile([P, 512], F32, tag="ot",
                                               name=f"ot{lb}_{dg}_{ls}")
                            if ls % 2 == 0:
                                nc.vector.tensor_scalar_mul(
                                    out=ot, in0=pss4[dg * 4 + ls],
                                    scalar1=rzs[ls])
                            else:
                                nc.scalar.activation(
                                    out=ot, in_=pss4[dg * 4 + ls],
                                    func=mybir.ActivationFunctionType.Copy,
                                    scale=rzs[ls])
                            r0 = lb * 512 + ls * P
                            nc.sync.dma_start(
                                out=out[r0:r0 + P, dg * 512:(dg + 1) * 512],
                                in_=ot)

    nc.compile()
    return nc


_NC_CACHE = {}


def _get_nc(zero_bias):
    if zero_bias not in _NC_CACHE:
        _NC_CACHE[zero_bias] = build_program(zero_bias)
    return _NC_CACHE[zero_bias]


def kernel(ix, iother, W, b):
    """Full-input entry point: shards batch across 8 NeuronCores."""
    from concourse.bass_utils import run_bass_kernel_spmd

    ix = np.ascontiguousarray(np.asarray(ix, dtype=np.float32))
    iother = np.ascontiguousarray(np.asarray(iother, dtype=np.float32))
    W = np.ascontiguousarray(np.asarray(W, dtype=np.float32))
    b = np.ascontiguousarray(np.asarray(b, dtype=np.float32))

    nc = _get_nc(zero_bias=bool(np.all(b == 0.0)))
    core_ids = list(range(NB))
    ident = np.eye(P, dtype=np.float32)
    in_maps = [
        {"ix": ix[i], "iother": iother[i], "W": W, "b": b, "ident": ident}
        for i in range(NB)
    ]
    res = run_bass_kernel_spmd(nc, in_maps, core_ids)
    outs = [res.results[i]["out"] for i in range(NB)]
    return np.stack(outs, axis=0).astype(np.float32)



# revision 49
# speedup vs baseline: 1.0619x; 1.0619x over previous
"""Trainium2 Bass kernel for nn_Aligner (cross-attention aligner).

Math (per batch element i):
    ex      = ix[i] @ W.T + b          # [L, D]
    eother  = iother[i] @ W.T + b      # [L, D]
    align   = softmax(ex @ eother.T)   # [L, L], softmax over last dim
    out[i]  = align @ iother[i]        # [L, D]

Shapes: B=8, L=2048, D=1024, fp32.  Sharding: batch-parallel, one batch
element per NeuronCore (8 cores), W/b replicated.  No collectives.

All matmuls run in float32r (full PE rate at N>=256).  TRN2 fp32r
rounds matmul inputs to 11 mantissa bits (RNE, HW-verified); engine
writes into f32r tiles round the same way.  An 11-bit logit pipeline is
~3e-2 off the fp32 reference, so precision is recovered via hi/lo
splits: x is stored as xh = rne11(x) (read exactly by the PE) plus
xl = rne11(x - xh), and X@Y = Xh@Yh (f32r) + [Xh@Yl + Xl@Yh].  The
bracketed cross terms (2^-12-scale corrections) are computed in ONE
fp8e4m3 perf_mode=DoubleRow pass at 0.5 cyc/row, interleaving
(Xh*2^-2, Xl*2^10) / (Yl*2^10, Yh*2^-2) pairs so both products carry a
2^8 scale, removed when merging the cross PSUM into the fp32 logits.

For b == 0 (this problem's inputs), align = ix @ G @ iother^T with
G = W^T@W (equal up to a softmax-invariant per-row shift), which
replaces the eother projection with a cheaper symmetric G compute:
G = Wh^T@Wh + C + C^T with C = Wh^T@Wl done once in bf16.  A generic
3-pass fallback program handles b != 0.

Measured: 729,161 ns/core (cost model; PE 77% busy), hardware
max-scale-relative error 3.9e-4 across all 8 batches.

Per-core dataflow:
  phase A: G (or WT hi/lo) in SBUF; ixG-projection + iother-transpose
           blocks interleaved, hi/lo pairs -> per-block DRAM scratch
  phase B, per l-block of 512 rows:
     S = f32r main + fp8-DoubleRow cross align into fp32 E tiles;
     softmax row-max split in halves (first half hidden under align),
     one exp with fused accum_out row-sum; E PE-transposed -> ET;
     out = ET.T @ iother tiles (dual-queue fed), scaled by 1/Z
"""

import numpy as np

import concourse.bass as bass
import concourse.mybir as mybir
import concourse.tile as tile
from concourse import bacc

P = 128          # partitions
L = 2048         # sequence length
D = 1024         # feature dim
NB = 8           # batch / cores
KC = D // P      # 8 contraction chunks for stage-1 matmuls
DG = D // P      # 8 output d-groups of stage 1
NLB = L // 512   # 4 l-blocks of 512
LS = 4           # l-subs of 128 per l-block
MC = L // 512    # 4 m-chunks of 512 for align
M16 = L // P     # 16 m-chunks of 128 for stage 4

F32 = mybir.dt.float32
F32R = mybir.dt.float32r
FP8 = mybir.dt.float8e4
BF16 = mybir.dt.bfloat16

# Include the C = Wh^T@Wl correction in G.  Dropping it costs ~4e-3
# max-scale output error (numpy-modelled; tolerance 2e-2) and saves the
# whole C pipeline in phase A.  Flip to True if hardware error surprises.
USE_C = False
DROW = mybir.MatmulPerfMode.DoubleRow
COPYF = mybir.ActivationFunctionType.Copy
AX = mybir.AxisListType.X
EXP = mybir.ActivationFunctionType.Exp


def build_program(zero_bias=False):
    """zero_bias=True uses the G = W^T@W factorization:
    ex@eother^T = ix@G@iother^T (+ bias terms that vanish for b=0, up to a
    softmax-invariant per-row shift).  This removes the eother projection
    and all W transposes; G costs half an eother projection."""
    nc = bacc.Bacc("TRN2", target_bir_lowering=False, debug=False)

    ix = nc.dram_tensor("ix", [L, D], F32, kind="ExternalInput").ap()
    iother = nc.dram_tensor("iother", [L, D], F32, kind="ExternalInput").ap()
    W = nc.dram_tensor("W", [D, D], F32, kind="ExternalInput").ap()
    bvec = nc.dram_tensor("b", [D], F32, kind="ExternalInput").ap()
    out = nc.dram_tensor("out", [L, D], F32, kind="ExternalOutput").ap()
    # identity for PE transpose-mode, fed from host (avoids f32r memset)
    ident_in = nc.dram_tensor("ident", [P, P], F32R, kind="ExternalInput").ap()

    # staging: projected-transposed activations (hi/lo), phase A -> phase B.
    # One DRAM tensor per 512-block so Tile's per-tensor DRAM dependency
    # tracking lets phase-B reads start as soon as their block is written.
    def scratch(name):
        t = nc.dram_tensor(name, [D, 512], F32R).ap()
        return t.rearrange("(dg p) l -> p dg l", p=P)           # [128, 8, 512]

    exT_h = [scratch(f"exTh_scratch{i}") for i in range(NLB)]
    exT_l = [scratch(f"exTl_scratch{i}") for i in range(NLB)]
    eoT_h = [scratch(f"eoTh_scratch{i}") for i in range(NLB)]

    # fp8 Xh*2^-13 / Xl*2^-4 words of exT, encoded at proj-eviction time
    # in phase A; phase B loads 1MB per l-block (replaces the 2MB exT-lo
    # f32r stream AND the per-l-block fp8 encode ops).  Slot-split into
    # two tensors to keep every DMA at <=3 non-partition dims.
    def scratch8(name):
        t = nc.dram_tensor(name, [D, 512], FP8).ap()
        return t.rearrange("(dg p) l -> p dg l", p=P)        # [128, 8, 512]

    exq8h_s = [scratch8(f"exq8h_scratch{i}") for i in range(NLB)]
    exq8l_s = [scratch8(f"exq8l_scratch{i}") for i in range(NLB)]
    # eoT lo exists only as the fp8 word inside the SBUF-resident eoq8 in
    # the zero-bias program; the generic program still stages it in DRAM.
    eoT_l = [scratch(f"eoTl_scratch{i}") for i in range(NLB)]

    with tile.TileContext(nc) as tc:
        with (
            tc.tile_pool(name="const", bufs=1) as const,
            tc.tile_pool(name="exTb", bufs=1) as exTb_pool,
            tc.tile_pool(name="eoTb", bufs=2) as eoTb_pool,
            tc.tile_pool(name="psum_tp", bufs=2, space="PSUM") as psum_tp,
            tc.tile_pool(name="psum_mm", bufs=2, space="PSUM") as psum_mm,
            tc.tile_pool(name="psum_s4", bufs=4, space="PSUM") as psum_s4,
        ):
            identr = const.tile([P, P], F32R, name="identr")
            nc.gpsimd.dma_start(out=identr, in_=ident_in)
            identf = identr.bitcast(F32)
            identb = const.tile([P, P], BF16, name="identb")
            nc.scalar.copy(out=identb, in_=identr)

            if zero_bias:
                # SBUF-resident fp8 (Yl*2^13, Yh*2^4) pairs of the FULL
                # eoT, built once during the phase-A transposes.  Phase B's
                # align cross pass reads slices directly -- no per-l-block
                # eoT-lo streaming and no per-chunk yq rebuilds.
                eoq8_sb = const.tile([P, DG, 2, L], FP8, name="eoq8_sb")

            # b laid out [p, dg]: btile[p, dg] = b[dg*128 + p]
            # (only the generic-bias path reads it)
            if not zero_bias:
                btile = const.tile([P, DG], F32)
                nc.sync.dma_start(out=btile,
                                  in_=bvec.rearrange("(c p) -> p c", p=P))

            def transpose_128_group(src_row, dst, rdtype=False, bf=False):
                """Transpose four [128,128] slices through one PSUM bank;
                single eviction into dst ([128, 4, 128] SBUF AP).
                rdtype=False: fp32 transpose-mode (bit-exact, 2 cyc/row);
                rdtype=True: f32r mode (rounds to 11 bits, 1.5 cyc/row);
                bf=True: bf16 in/out (1 cyc/row)."""
                dt = BF16 if bf else (F32R if rdtype else F32)
                idt = identb if bf else (identr if rdtype else identf)
                tp = psum_tp.tile([P, 4 * P], dt, tag="tp", name="tpg")
                for i in range(4):
                    nc.tensor.transpose(
                        tp[:, i * P:(i + 1) * P],
                        src_row[:, i * P:(i + 1) * P],
                        idt,
                    )
                nc.scalar.copy(out=dst, in_=tp.rearrange(
                    "p (four c) -> p four c", four=4))

            _tp_rr = [0]

            def transpose_128_group_hl(src_row, dst_h, dst_l, borrow=False):
                """Like transpose_128_group, but evicts an f32r hi/lo pair:
                hi = rne11(psum) via ACT/DVE copy (alternating), lo = psum -
                hi via DVE sub.  borrow=True also rotates through the
                (phase-A-idle) stage-4 PSUM banks for a deeper transpose
                pipeline."""
                _tp_rr[0] += 1
                if borrow and _tp_rr[0] % 3 != 0:
                    tp = psum_s4.tile([P, 4 * P], F32, tag="s4",
                                      name=f"tpb{_tp_rr[0]}")
                else:
                    tp = psum_tp.tile([P, 4 * P], F32, tag="tp",
                                      name=f"tpt{_tp_rr[0]}")
                for i in range(4):
                    nc.tensor.transpose(
                        tp[:, i * P:(i + 1) * P],
                        src_row[:, i * P:(i + 1) * P],
                        identf,
                    )
                tp4 = tp.rearrange("p (four c) -> p four c", four=4)
                if _tp_rr[0] % 2 == 0:
                    nc.scalar.copy(out=dst_h, in_=tp4)
                else:
                    nc.vector.tensor_copy(out=dst_h, in_=tp4)
                nc.vector.tensor_sub(out=dst_l, in0=tp4, in1=dst_h)

            # ---------------- phase A: WTh/WTl + exT/eoT (hi/lo) -> DRAM ----
            with (
                tc.tile_pool(name="wt", bufs=1) as wt_pool,
                tc.tile_pool(name="stage", bufs=3) as stage_pool,
                tc.tile_pool(name="ev", bufs=1) as ev_pool,
                tc.tile_pool(name="evt", bufs=2) as evt_pool,
            ):
                # lhsT hi/lo pair for the ix projection:
                #   direct path: WT (transposed W);  G path: G = W^T@W
                #   (symmetric, so its [i-part, j] layout is its own lhsT)
                wth = wt_pool.tile([P, KC, D], F32R)
                if zero_bias:
                    # G-lo lives only as the fp8 word in wq8 (slot 1); no
                    # f32r wtl tile is kept in this mode.
                    wq8 = wt_pool.tile([P, KC, 2, D], FP8, name="wq8")
                else:
                    wtl = wt_pool.tile([P, KC, D], F32R)

                if zero_bias:
                  # G = W^T@W via hi/lo.  G is symmetric, so the two cross
                  # terms are each other's transposes: Wh^T@Wl = (Wl^T@Wh)^T.
                  # C = Wh^T@Wl is a 2^-12-scale correction, so it runs in
                  # pure bf16 (err ~2^-21 * G).  C^T is also lo-scale, so it
                  # is added into the LO part after the hi/lo split, reading
                  # the transpose PSUM directly -- no C^T SBUF tensor.
                  with (
                      tc.tile_pool(name="split", bufs=2) as split_pool,
                      tc.tile_pool(name="cpool", bufs=1) as c_pool,
                  ):
                    if USE_C:
                        cmat = c_pool.tile([P, KC, D], BF16, name="cmat")
                        # fp8 caches of Wh*2^4 / Wl*2^16, laid out as
                        # DoubleRow pairs over adjacent 128-row contraction
                        # chunks: the C pass runs at 0.5 cyc/row.  C is a
                        # 2^-12-scale correction, so fp8's 2^-4 word
                        # precision gives C to 2^-16 overall.  Product
                        # scale 2^20 is removed at the cmat eviction.
                        whc8 = c_pool.tile([P, KC // 2, 2, D], FP8,
                                           name="whc8")
                        wlc8 = c_pool.tile([P, KC // 2, 2, D], FP8,
                                           name="wlc8")

                    def g_psums(pfx):
                        return ([psum_mm.tile([P, 512], F32, tag="mm",
                                              name=f"{pfx}_{i}")
                                 for i in range(2)]
                                + [psum_s4.tile([P, 512], F32, tag="s4",
                                                name=f"{pfx}_{i + 2}")
                                   for i in range(4)]
                                + [psum_tp.tile([P, 512], F32, tag="tp",
                                                name=f"{pfx}_{i + 6}")
                                   for i in range(2)])

                    # fp8 pair caches of Wh / Wl
                    for dc in range(DG if USE_C else 0):
                        wrow = stage_pool.tile([P, D], F32, tag="stage",
                                               name=f"gw{dc}")
                        nc.sync.dma_start(
                            out=wrow, in_=W[dc * P:(dc + 1) * P, :])
                        whr = split_pool.tile([P, D], F32R, tag="whc",
                                              name=f"gwh{dc}")
                        nc.vector.tensor_copy(out=whr, in_=wrow)
                        nc.scalar.activation(
                            out=whc8[:, dc // 2, dc % 2, :], in_=wrow,
                            func=COPYF, scale=16.0)
                        wl32 = split_pool.tile([P, D], F32, tag="wl32",
                                               name=f"gwl{dc}")
                        nc.vector.tensor_sub(out=wl32, in0=wrow, in1=whr)
                        nc.scalar.activation(
                            out=wlc8[:, dc // 2, dc % 2, :], in_=wl32,
                            func=COPYF, scale=65536.0)

                    # C = Wh^T @ Wl, fp8 DoubleRow (pairs = adjacent chunks)
                    for jh in range(2 if USE_C else 0):
                        jsl = slice(jh * 512, (jh + 1) * 512)
                        pss = g_psums(f"c{jh}")
                        for dp in range(DG // 2):
                            for ic in range(DG):
                                nc.tensor.matmul(
                                    pss[ic],
                                    whc8[:, dp, :, ic * P:(ic + 1) * P],
                                    wlc8[:, dp, :, jsl],
                                    start=(dp == 0), stop=(dp == DG // 2 - 1),
                                    perf_mode=DROW)
                        for ic in range(DG):
                            nc.scalar.activation(
                                out=cmat[:, ic, jsl], in_=pss[ic],
                                func=COPYF, scale=2.0 ** -20)

                    # G = Wh^T@Wh + C + C^T; Wh streamed per (jh, dc)
                    for jh in range(2):
                        jsl = slice(jh * 512, (jh + 1) * 512)
                        pss = g_psums(f"g{jh}")
                        for dc in range(DG):
                            # DMA W rows straight into an f32r tile (bit
                            # pattern is fp32; the PE read rounds to 11
                            # bits) -- no DVE conversion copy on the
                            # matmul critical path
                            whc = split_pool.tile([P, D], F32R, tag="whc",
                                                  name=f"g2wh{jh}_{dc}")
                            nc.sync.dma_start(
                                out=whc,
                                in_=W[dc * P:(dc + 1) * P, :].bitcast(F32R))
                            for ic in range(DG):
                                nc.tensor.matmul(
                                    pss[ic], whc[:, ic * P:(ic + 1) * P],
                                    whc[:, jsl],
                                    start=(dc == 0), stop=(dc == DG - 1))
                        for ic in range(DG):
                            if USE_C:
                                tmp = split_pool.tile(
                                    [P, 512], F32, tag="gtmp",
                                    name=f"ga{jh}_{ic}")
                                nc.vector.tensor_add(out=tmp, in0=pss[ic],
                                                     in1=cmat[:, ic, jsl])
                            else:
                                tmp = pss[ic]
                            nc.scalar.copy(out=wth[:, ic, jsl], in_=tmp)
                            gl = split_pool.tile([P, 512], F32, tag="gl",
                                                 name=f"gl{jh}_{ic}")
                            nc.vector.tensor_sub(out=gl, in0=tmp,
                                                 in1=wth[:, ic, jsl])
                            if USE_C:
                                # C^T via PE transposes of cmat, read from
                                # PSUM.  Allocated from psum_mm (not
                                # psum_tp): the tp slots are held by
                                # pss[6]/pss[7] until the last evictions,
                                # and the gl/gtmp slot chain feeds back
                                # into ctp -- psum_tp here would deadlock.
                                ctp = psum_mm.tile([P, 4 * P], BF16,
                                                   tag="mm",
                                                   name=f"ctp{jh}_{ic}")
                                for t in range(4):
                                    jc = jh * 4 + t
                                    nc.tensor.transpose(
                                        ctp[:, t * P:(t + 1) * P],
                                        cmat[:, jc, ic * P:(ic + 1) * P],
                                        identb)
                                nc.vector.tensor_add(
                                    out=gl, in0=gl,
                                    in1=ctp.rearrange(
                                        "p (four c) -> p four c", four=4))
                            # G-lo straight to its fp8 DoubleRow word
                            # (Gl*2^13); pairs with xh*2^-13 in the proj
                            # cross pass so both slot products land at
                            # scale 2^0 vs the f32r hi*hi pass and the
                            # cross pass accumulates into the SAME psum.
                            nc.scalar.activation(out=wq8[:, ic, 1, jsl],
                                                 in_=gl, func=COPYF,
                                                 scale=8192.0)
                    # hi word: Gh*2^2, pairs with xl*2^-2
                    nc.scalar.activation(out=wq8[:, :, 0, :], in_=wth,
                                         func=COPYF, scale=4.0)
                else:
                    for dc in range(DG):
                        wrow = stage_pool.tile([P, D], F32, tag="stage",
                                               name=f"wrow{dc}")
                        nc.sync.dma_start(out=wrow,
                                          in_=W[dc * P:(dc + 1) * P, :])
                        for q in range(2):
                            transpose_128_group_hl(
                                wrow[:, q * 4 * P:(q + 1) * 4 * P],
                                wth[:, q * 4:(q + 1) * 4, dc * P:(dc + 1) * P],
                                wtl[:, q * 4:(q + 1) * 4, dc * P:(dc + 1) * P],
                                borrow=True)

                with (
                    tc.tile_pool(name="xT", bufs=1) as xT_pool,
                    tc.tile_pool(name="ev", bufs=1) as ev_pool,
                    tc.tile_pool(name="evt", bufs=2) as evt_pool,
                    tc.tile_pool(name="iotp", bufs=2) as iotp_pool,
                ):
                    def tp_block(src_dram, dst_h, dst_l, pfx, blk):
                        """dst = src_blk^T (hi/lo split), via small per-group
                        tiles DMA'd out immediately -- fills PE bubbles between
                        projection blocks without big-slot contention.
                        zero_bias: hi also goes to DRAM (SWDGE queue -- the
                        sync queue is phase A's bottleneck), lo goes ONLY
                        into the resident eoq8 fp8 pairs."""
                        for s in range(4):
                            row = stage_pool.tile([P, D], F32, tag="stage",
                                                  name=f"{pfx}row{blk}_{s}")
                            r0 = (blk * 4 + s) * P
                            nc.sync.dma_start(out=row, in_=src_dram[r0:r0 + P, :])
                            ssl = slice(s * P, (s + 1) * P)
                            for q in range(2):
                                th = iotp_pool.tile([P, 4, P], F32R, tag="ioh",
                                                    name=f"{pfx}h{blk}_{s}_{q}")
                                tl = iotp_pool.tile([P, 4, P], F32R, tag="iol",
                                                    name=f"{pfx}l{blk}_{s}_{q}")
                                transpose_128_group_hl(
                                    row[:, q * 4 * P:(q + 1) * 4 * P], th, tl)
                                qsl = slice(q * 4, (q + 1) * 4)
                                if zero_bias:
                                    nc.gpsimd.dma_start(
                                        out=dst_h[blk][:, qsl, ssl], in_=th)
                                    lr = slice(blk * 512 + s * P,
                                               blk * 512 + (s + 1) * P)
                                    nc.scalar.activation(
                                        out=eoq8_sb[:, qsl, 1, lr], in_=th,
                                        func=COPYF, scale=16.0)
                                    nc.vector.tensor_scalar_mul(
                                        out=eoq8_sb[:, qsl, 0, lr], in0=tl,
                                        scalar1=8192.0)
                                else:
                                    nc.sync.dma_start(
                                        out=dst_h[blk][:, qsl, ssl], in_=th)
                                    nc.sync.dma_start(
                                        out=dst_l[blk][:, qsl, ssl], in_=tl)

                    def proj_block(src_dram, dst_h, dst_l, pfx, blk):
                        """dst[blk] = lhsT_pair @ src_blk^T + b, stored
                        hi/lo.  zero_bias: x is transposed with a 4096*I
                        identity (x carries 2^12), the hi*hi pass runs in
                        f32r and the two cross terms run in ONE fp8
                        DoubleRow pass accumulating into the same psum.
                        Generic path: 3-pass f32r hi/lo.  Processed as two
                        256-halves with double-buffered xh/xl so half h+1's
                        transposes and evictions overlap half h's matmuls."""
                        for hf in range(2):
                            xh = xT_pool.tile([P, KC, 256], F32R, tag="xh",
                                              bufs=2, name=f"{pfx}xh{blk}_{hf}")
                            xl = xT_pool.tile([P, KC, 256], F32R, tag="xl",
                                              bufs=2, name=f"{pfx}xl{blk}_{hf}")
                            for si in range(2):
                                s_ = hf * 2 + si
                                row = stage_pool.tile(
                                    [P, D], F32, tag="stage",
                                    name=f"{pfx}row{blk}_{s_}")
                                r0 = (blk * 4 + s_) * P
                                nc.sync.dma_start(
                                    out=row, in_=src_dram[r0:r0 + P, :])
                                if zero_bias:
                                    # x carries 2^12 through proj and align
                                    # (see wq8 comment); exact 2^k scaling,
                                    # in place before the transpose
                                    nc.scalar.activation(
                                        out=row, in_=row, func=COPYF,
                                        scale=4096.0)
                                ssl = slice(si * P, (si + 1) * P)
                                for q in range(2):
                                    transpose_128_group_hl(
                                        row[:, q * 4 * P:(q + 1) * 4 * P],
                                        xh[:, q * 4:(q + 1) * 4, ssl],
                                        xl[:, q * 4:(q + 1) * 4, ssl],
                                        borrow=True)
                            if zero_bias:
                                xq8 = xT_pool.tile(
                                    [P, KC, 2, 256], FP8, tag="xq8", bufs=2,
                                    name=f"{pfx}xq8{blk}_{hf}")
                                q8h = ev_pool.tile(
                                    [P, DG, 256], FP8, tag="q8h", bufs=2,
                                    name=f"{pfx}q8h{blk}_{hf}")
                                q8l = ev_pool.tile(
                                    [P, DG, 256], FP8, tag="q8l", bufs=2,
                                    name=f"{pfx}q8l{blk}_{hf}")
                                nc.vector.tensor_scalar_mul(
                                    out=xq8[:, :, 0, :], in0=xl, scalar1=0.25)
                                nc.scalar.activation(
                                    out=xq8[:, :, 1, :], in_=xh, func=COPYF,
                                    scale=2.0 ** -13)
                            hsl = slice(hf * 256, (hf + 1) * 256)
                            for dg in range(DG):
                                if dg % 2 == 0:
                                    evh = ev_pool.tile(
                                        [P, 2, 256], F32R, tag="evh", bufs=2,
                                        name=f"{pfx}evh{blk}_{hf}_{dg}")
                                    evl = ev_pool.tile(
                                        [P, 2, 256], F32R, tag="evl", bufs=2,
                                        name=f"{pfx}evl{blk}_{hf}_{dg}")
                                if zero_bias:
                                    # two dg share one [P, 2, 256] psum
                                    # bank so the hi/lo eviction runs as a
                                    # single [P, 512] op pair
                                    if dg % 2 == 0:
                                        ps2 = psum_mm.tile(
                                            [P, 2, 256], F32, tag="mm",
                                            name=f"{pfx}ps{blk}_{hf}_{dg}")
                                    ps = ps2[:, dg % 2, :]
                                    for kc in range(KC):
                                        nc.tensor.matmul(
                                            ps,
                                            wth[:, kc, dg * P:(dg + 1) * P],
                                            xh[:, kc, :],
                                            start=(kc == 0), stop=False,
                                        )
                                    for kc in range(KC):
                                        nc.tensor.matmul(
                                            ps,
                                            wq8[:, kc, :, dg * P:(dg + 1) * P],
                                            xq8[:, kc, :, :],
                                            start=False, stop=(kc == KC - 1),
                                            perf_mode=DROW,
                                        )
                                else:
                                    ps = psum_mm.tile(
                                        [P, 256], F32, tag="mm",
                                        name=f"{pfx}ps{blk}_{hf}_{dg}")
                                    n = 0
                                    for wt_t, x_t in ((wth, xh), (wth, xl),
                                                      (wtl, xh)):
                                        for kc in range(KC):
                                            nc.tensor.matmul(
                                                ps,
                                                wt_t[:, kc, dg * P:(dg + 1) * P],
                                                x_t[:, kc, :],
                                                start=(n == 0),
                                                stop=(n == 3 * KC - 1),
                                            )
                                            n += 1
                                if zero_bias:
                                    if dg % 2 == 1:
                                        nc.scalar.copy(out=evh, in_=ps2)
                                        nc.vector.tensor_sub(
                                            out=evl, in0=ps2, in1=evh)
                                        # fp8 align-cross words, encoded
                                        # here (phase B just loads them)
                                        nc.scalar.activation(
                                            out=q8h[:, dg - 1:dg + 1, :],
                                            in_=evh,
                                            func=COPYF, scale=2.0 ** -13)
                                        nc.vector.tensor_scalar_mul(
                                            out=q8l[:, dg - 1:dg + 1, :],
                                            in0=evl,
                                            scalar1=2.0 ** -4)
                                else:
                                    tmp = evt_pool.tile(
                                        [P, 256], F32, tag="evt",
                                        name=f"{pfx}tmp{blk}_{hf}_{dg}")
                                    nc.vector.tensor_scalar_add(
                                        out=tmp, in0=ps,
                                        scalar1=btile[:, dg:dg + 1])
                                    nc.vector.tensor_copy(
                                        out=evh[:, dg % 2, :], in_=tmp)
                                    nc.vector.tensor_sub(
                                        out=evl[:, dg % 2, :], in0=tmp,
                                        in1=evh[:, dg % 2, :])
                                if dg % 2 == 1:
                                    dsl = slice(dg - 1, dg + 1)
                                    nc.sync.dma_start(
                                        out=dst_h[blk][:, dsl, hsl], in_=evh)
                                    if not zero_bias:
                                        # zero_bias keeps lo only as the
                                        # fp8 words (exq8*_s)
                                        nc.sync.dma_start(
                                            out=dst_l[blk][:, dsl, hsl],
                                            in_=evl)
                            if zero_bias:
                                nc.gpsimd.dma_start(
                                    out=exq8h_s[blk][:, :, hsl], in_=q8h)
                                nc.gpsimd.dma_start(
                                    out=exq8l_s[blk][:, :, hsl], in_=q8l)

                    if zero_bias:
                        # ALL io-transpose blocks first: phase B's first
                        # align chunk needs the complete eoq8_sb, and the
                        # transposes also fill the PE while the G evictions
                        # drain on ACT/DVE.  proj blocks follow; phase B's
                        # lb=0 can start right after proj block 0 lands.
                        for blk in range(NLB):
                            tp_block(iother, eoT_h, eoT_l, "eo", blk)
                        for blk in range(NLB):
                            proj_block(ix, exT_h, exT_l, "ex", blk)
                    else:
                        for blk in range(NLB):
                            proj_block(ix, exT_h, exT_l, "ex", blk)
                        for blk in range(NLB):
                            proj_block(iother, eoT_h, eoT_l, "eo", blk)

            # ---------------- phase B: align + softmax + output -------------
            with (
                tc.tile_pool(name="epool", bufs=4) as e_pool,
                tc.tile_pool(name="q8", bufs=1) as q8_pool,
                tc.tile_pool(name="c32", bufs=4) as c32_pool,
                tc.tile_pool(name="etpool", bufs=4) as et_pool,
                tc.tile_pool(name="s4rhs", bufs=6) as s4rhs_pool,
                tc.tile_pool(name="outp", bufs=6) as out_pool,
                tc.tile_pool(name="small", bufs=10) as small_pool,
            ):
                for lb in range(NLB):
                    exbh = exTb_pool.tile([P, DG, 512], F32R, tag="exbh",
                                          name=f"exbh{lb}")
                    # SWDGE queue: lets these overtake phase-A writes still
                    # pending in the sync-engine HWDGE FIFO
                    nc.gpsimd.dma_start(out=exbh, in_=exT_h[lb])
                    if not zero_bias:
                        exbl = exTb_pool.tile([P, DG, 512], F32R, tag="exbl",
                                              name=f"exbl{lb}")
                        nc.gpsimd.dma_start(out=exbl, in_=exT_l[lb])

                    NMC = 2 * MC      # 8 chunks of 256
                    Es = [e_pool.tile([P, L], F32, tag="E",
                                      name=f"E{lb}_{i}") for i in range(LS)]
                    nms = {}
                    if zero_bias:
                        # fp8 interleaved (Xh*2^-13, Xl*2^-4) pairs of exT
                        # (X carries 2^12), encoded in phase A -- just load.
                        # Paired with y words (Yl*2^13, Yh*2^4) both slot
                        # products land at scale 2^0 vs the f32r hi*hi pass,
                        # so the cross pass accumulates into the same psum.
                        exq8 = q8_pool.tile([P, 2, DG, 512], FP8, tag="exq8",
                                            name=f"exq8{lb}")
                        nc.gpsimd.dma_start(out=exq8[:, 0, :, :],
                                            in_=exq8h_s[lb])
                        nc.gpsimd.dma_start(out=exq8[:, 1, :, :],
                                            in_=exq8l_s[lb])
                    for mc in range(NMC):
                        msl = slice(mc * 256, (mc + 1) * 256)
                        blk_i, half = mc // 2, mc % 2
                        hsl = slice(half * 256, (half + 1) * 256)
                        eobh = eoTb_pool.tile([P, DG, 256], F32R, tag="eobh",
                                              name=f"eobh{lb}_{mc}")
                        nc.gpsimd.dma_start(out=eobh,
                                            in_=eoT_h[blk_i][:, :, hsl])
                        if zero_bias:
                            # fp8 (Yl*2^13, Yh*2^4) pairs come straight from
                            # the resident eoq8_sb -- no lo stream, no build
                            yq = eoq8_sb[:, :, :, msl]
                        else:
                            eobl = eoTb_pool.tile([P, DG, 256], F32R,
                                                  tag="eobl",
                                                  name=f"eobl{lb}_{mc}")
                            nc.gpsimd.dma_start(out=eobl,
                                                in_=eoT_l[blk_i][:, :, hsl])
                        for ls in range(LS):
                            xsl = slice(ls * P, (ls + 1) * P)
                            if zero_bias:
                                # hi*hi f32r pass + cross terms Xh@Yl+Xl@Yh
                                # in ONE fp8 DoubleRow continuation of the
                                # SAME psum accumulation (scales match)
                                ps = psum_mm.tile([P, 256], F32, tag="mm",
                                                  name=f"al{lb}_{mc}_{ls}")
                                for dc in range(DG):
                                    nc.tensor.matmul(
                                        ps, exbh[:, dc, xsl], eobh[:, dc, :],
                                        start=(dc == 0), stop=False,
                                    )
                                for dc in range(DG):
                                    nc.tensor.matmul(
                                        ps,
                                        exq8[:, :, dc, xsl],
                                        yq[:, dc, :, :],
                                        start=False,
                                        stop=(dc == DG - 1),
                                        perf_mode=DROW,
                                    )
                                # E is fp32: store raw 2^12-scaled logits;
                                # the PSUM is freed after this single read
                                # (alternate engines to balance load)
                                if (mc + ls) % 2 == 0:
                                    nc.vector.tensor_copy(
                                        out=Es[ls][:, msl], in_=ps)
                                else:
                                    nc.scalar.copy(
                                        out=Es[ls][:, msl], in_=ps)
                                if mc == 3:
                                    # first-half row max, hidden under the
                                    # align of chunks 4-7
                                    nms[ls] = small_pool.tile(
                                        [P, 1], F32, tag="nm1",
                                        name=f"nm1_{lb}_{ls}")
                                    nc.vector.reduce_max(
                                        nms[ls], Es[ls][:, :1024], axis=AX,
                                        negate=True)
                                continue
                            ps = psum_mm.tile([P, 256], F32, tag="mm",
                                              name=f"al{lb}_{mc}_{ls}")
                            n = 0
                            for x_t, eo_t in ((exbh, eobh), (exbh, eobl),
                                              (exbl, eobh)):
                                for dc in range(DG):
                                    nc.tensor.matmul(
                                        ps,
                                        x_t[:, dc, ls * P:(ls + 1) * P],
                                        eo_t[:, dc, :],
                                        start=(n == 0), stop=(n == 3 * DG - 1),
                                    )
                                    n += 1
                            nc.vector.tensor_copy(out=Es[ls][:, msl],
                                                    in_=ps)

                    ets = []
                    rzs = []
                    for ls in range(LS):
                        E = Es[ls]
                        negM = small_pool.tile([P, 1], F32, tag="negM",
                                               name=f"nm{lb}_{ls}")
                        if zero_bias:
                            nc.vector.reduce_max(negM, E[:, 1024:], axis=AX,
                                                 negate=True)
                            # -max(a,b) = min(-a,-b)
                            nc.vector.tensor_tensor(
                                out=negM, in0=negM, in1=nms[ls],
                                op=mybir.AluOpType.min)
                        else:
                            nc.vector.reduce_max(negM, E, axis=AX,
                                                 negate=True)
                        zsum = small_pool.tile([P, 1], F32, tag="zsum",
                                               name=f"zs{lb}_{ls}")
                        if zero_bias:
                            # logits carry 2^12; EXP's scale knob removes it
                            # (bias must then be -max * 2^-12 as well).
                            # exp output goes to an f32r tile: 1.5
                            # cyc/row E-transposes instead of fp32's 2
                            # (bf16 at 1.0 is rejected by the NEFF
                            # compiler when mixed with the f32r stage-4
                            # rhs: "Mixing of 32-bit and non-32-bit
                            # Matmult inputs not supported").
                            negMs = small_pool.tile([P, 1], F32, tag="negMs",
                                                    name=f"nms{lb}_{ls}")
                            nc.vector.tensor_scalar_mul(
                                out=negMs, in0=negM, scalar1=2.0 ** -12)
                            E2 = e_pool.tile([P, L], F32R, tag="E2",
                                             bufs=2, name=f"E2_{lb}_{ls}")
                            nc.scalar.activation(
                                out=E2, in_=E, func=EXP, bias=negMs,
                                scale=2.0 ** -12, accum_out=zsum)
                        else:
                            nc.scalar.activation(
                                out=E, in_=E, func=EXP, bias=negM, scale=1.0,
                                accum_out=zsum)
                        rz = small_pool.tile([P, 1], F32, tag="rz",
                                             name=f"rz{lb}_{ls}")
                        nc.vector.reciprocal(rz, zsum)
                        rzs.append(rz)
                        # ET[p, m16, l] = E[l, m16*128 + p]
                        ET = et_pool.tile([P, M16, P], F32R,
                                          tag="ET", name=f"ET{lb}_{ls}")
                        for q in range(4):
                            transpose_128_group(
                                (E2 if zero_bias else E)[:, q * 4 * P:
                                                         (q + 1) * 4 * P],
                                ET[:, q * 4:(q + 1) * 4, :],
                                rdtype=zero_bias)
                        ets.append(ET)

                    # stage 4: out rows = (E @ iother) * rz.  One [P, 1024]
                    # rhs load per m16 feeds BOTH d-halves (half the DMA
                    # dispatches); 8 psum banks (s4 + borrowed mm/tp, all
                    # idle here) hold the full 512x1024 output block.
                    pss4 = (
                        [psum_s4.tile([P, 512], F32, tag="s4",
                                      name=f"s4_{lb}_{i}") for i in range(4)]
                        + [psum_mm.tile([P, 512], F32, tag="mm",
                                        name=f"s4m_{lb}_{i}") for i in range(2)]
                        + [psum_tp.tile([P, 512], F32, tag="tp",
                                        name=f"s4t_{lb}_{i}") for i in range(2)]
                    )
                    for m16 in range(M16):
                        rhs = s4rhs_pool.tile([P, D], F32R, tag="s4rhs",
                                              name=f"rhs{lb}_{m16}")
                        # feed the wave from BOTH DMA queues: one queue
                        # alone cannot keep up with the PE
                        eng = nc.sync if m16 % 2 == 0 else nc.gpsimd
                        eng.dma_start(
                            out=rhs,
                            in_=iother[m16 * P:(m16 + 1) * P, :].bitcast(F32R))
                        for dg in range(2):
                            for ls in range(LS):
                                nc.tensor.matmul(
                                    pss4[dg * 4 + ls],
                                    ets[ls][:, m16, :],
                                    rhs[:, dg * 512:(dg + 1) * 512],
                                    start=(m16 == 0), stop=(m16 == M16 - 1),
                                )
                    for dg in range(2):
                        for ls in range(LS):
                            ot = out_pool.tile([P, 512], F32, tag="ot",
                                               name=f"ot{lb}_{dg}_{ls}")
                            if ls % 2 == 0:
                                nc.vector.tensor_scalar_mul(
                                    out=ot, in0=pss4[dg * 4 + ls],
                                    scalar1=rzs[ls])
                            else:
                                nc.scalar.activation(
                                    out=ot, in_=pss4[dg * 4 + ls],
                                    func=mybir.ActivationFunctionType.Copy,
                                    scale=rzs[ls])
                            r0 = lb * 512 + ls * P
                            nc.sync.dma_start(
                                out=out[r0:r0 + P, dg * 512:(dg + 1) * 512],
                                in_=ot)

    nc.compile()
    return nc


_NC_CACHE = {}


def _get_nc(zero_bias):
    if zero_bias not in _NC_CACHE:
        _NC_CACHE[zero_bias] = build_program(zero_bias)
    return _NC_CACHE[zero_bias]


def kernel(ix, iother, W, b):
    """Full-input entry point: shards batch across 8 NeuronCores."""
    from concourse.bass_utils import run_bass_kernel_spmd

    ix = np.ascontiguousarray(np.asarray(ix, dtype=np.float32))
    iother = np.ascontiguousarray(np.asarray(iother, dtype=np.float32))
    W = np.ascontiguousarray(np.asarray(W, dtype=np.float32))
    b = np.ascontiguousarray(np.asarray(b, dtype=np.float32))

    nc = _get_nc(zero_bias=bool(np.all(b == 0.0)))
    core_ids = list(range(NB))
    ident = np.eye(P, dtype=np.float32)
    in_maps = [
        {"ix": ix[i], "iother": iother[i], "W": W, "b": b, "ident": ident}
        for i in range(NB)
    ]
    res = run_bass_kernel_spmd(nc, in_maps, core_ids)
    outs = [res.results[i]["out"] for i in range(NB)]
    return np.stack(outs, axis=0).astype(np.float32)



# revision 54
# speedup vs baseline: 1.1321x; 1.0662x over previous
"""Trainium2 Bass kernel for nn_Aligner (cross-attention aligner).

Math (per batch element i):
    ex      = ix[i] @ W.T + b          # [L, D]
    eother  = iother[i] @ W.T + b      # [L, D]
    align   = softmax(ex @ eother.T)   # [L, L], softmax over last dim
    out[i]  = align @ iother[i]        # [L, D]

Shapes: B=8, L=2048, D=1024, fp32.  Sharding: batch-parallel, one batch
element per NeuronCore (8 cores), W/b replicated.  No collectives.

All matmuls run in float32r (full PE rate at N>=256).  TRN2 fp32r
rounds matmul inputs to 11 mantissa bits (RNE, HW-verified); engine
writes into f32r tiles round the same way.  An 11-bit logit pipeline is
~3e-2 off the fp32 reference, so precision is recovered via hi/lo
splits: x is stored as xh = rne11(x) (read exactly by the PE) plus
xl = rne11(x - xh), and X@Y = Xh@Yh (f32r) + [Xh@Yl + Xl@Yh].  The
bracketed cross terms (2^-12-scale corrections) are computed in ONE
fp8e4m3 perf_mode=DoubleRow pass at 0.5 cyc/row, interleaving
(Xh*2^-2, Xl*2^10) / (Yl*2^10, Yh*2^-2) pairs so both products carry a
2^8 scale, removed when merging the cross PSUM into the fp32 logits.

For b == 0 (this problem's inputs), align = ix @ G @ iother^T with
G = W^T@W (equal up to a softmax-invariant per-row shift), which
replaces the eother projection with a cheaper symmetric G compute:
G = Wh^T@Wh + C + C^T with C = Wh^T@Wl done once in bf16.  A generic
3-pass fallback program handles b != 0.

Measured: 729,161 ns/core (cost model; PE 77% busy), hardware
max-scale-relative error 3.9e-4 across all 8 batches.

Per-core dataflow:
  phase A: G (or WT hi/lo) in SBUF; ixG-projection + iother-transpose
           blocks interleaved, hi/lo pairs -> per-block DRAM scratch
  phase B, per l-block of 512 rows:
     S = f32r main + fp8-DoubleRow cross align into fp32 E tiles;
     softmax row-max split in halves (first half hidden under align),
     one exp with fused accum_out row-sum; E PE-transposed -> ET;
     out = ET.T @ iother tiles (dual-queue fed), scaled by 1/Z
"""

import numpy as np

import concourse.bass as bass
import concourse.mybir as mybir
import concourse.tile as tile
from concourse import bacc

P = 128          # partitions
L = 2048         # sequence length
D = 1024         # feature dim
NB = 8           # batch / cores
KC = D // P      # 8 contraction chunks for stage-1 matmuls
DG = D // P      # 8 output d-groups of stage 1
NLB = L // 512   # 4 l-blocks of 512
LS = 4           # l-subs of 128 per l-block
MC = L // 512    # 4 m-chunks of 512 for align
M16 = L // P     # 16 m-chunks of 128 for stage 4

F32 = mybir.dt.float32
F32R = mybir.dt.float32r
FP8 = mybir.dt.float8e4
BF16 = mybir.dt.bfloat16

# Include the C = Wh^T@Wl correction in G.  Dropping it costs ~4e-3
# max-scale output error (numpy-modelled; tolerance 2e-2) and saves the
# whole C pipeline in phase A.  Flip to True if hardware error surprises.
USE_C = False
DROW = mybir.MatmulPerfMode.DoubleRow
COPYF = mybir.ActivationFunctionType.Copy
AX = mybir.AxisListType.X
EXP = mybir.ActivationFunctionType.Exp


def build_program(zero_bias=False):
    """zero_bias=True uses the G = W^T@W factorization:
    ex@eother^T = ix@G@iother^T (+ bias terms that vanish for b=0, up to a
    softmax-invariant per-row shift).  This removes the eother projection
    and all W transposes; G costs half an eother projection."""
    nc = bacc.Bacc("TRN2", target_bir_lowering=False, debug=False)

    ix = nc.dram_tensor("ix", [L, D], F32, kind="ExternalInput").ap()
    iother = nc.dram_tensor("iother", [L, D], F32, kind="ExternalInput").ap()
    W = nc.dram_tensor("W", [D, D], F32, kind="ExternalInput").ap()
    bvec = nc.dram_tensor("b", [D], F32, kind="ExternalInput").ap()
    out = nc.dram_tensor("out", [L, D], F32, kind="ExternalOutput").ap()
    # identity for PE transpose-mode, fed from host (avoids f32r memset)
    ident_in = nc.dram_tensor("ident", [P, P], F32R, kind="ExternalInput").ap()

    # staging: projected-transposed activations (hi/lo), phase A -> phase B.
    # One DRAM tensor per 512-block so Tile's per-tensor DRAM dependency
    # tracking lets phase-B reads start as soon as their block is written.
    def scratch(name):
        t = nc.dram_tensor(name, [D, 512], F32R).ap()
        return t.rearrange("(dg p) l -> p dg l", p=P)           # [128, 8, 512]

    exT_h = [scratch(f"exTh_scratch{i}") for i in range(NLB)]
    exT_l = [scratch(f"exTl_scratch{i}") for i in range(NLB)]
    eoT_h = [scratch(f"eoTh_scratch{i}") for i in range(NLB)]

    # fp8 Xh*2^-13 / Xl*2^-4 words of exT, encoded at proj-eviction time
    # in phase A; phase B loads 1MB per l-block (replaces the 2MB exT-lo
    # f32r stream AND the per-l-block fp8 encode ops).  Slot-split into
    # two tensors to keep every DMA at <=3 non-partition dims.
    def scratch8(name):
        t = nc.dram_tensor(name, [D, 512], FP8).ap()
        return t.rearrange("(dg p) l -> p dg l", p=P)        # [128, 8, 512]

    exq8h_s = [scratch8(f"exq8h_scratch{i}") for i in range(NLB)]
    exq8l_s = [scratch8(f"exq8l_scratch{i}") for i in range(NLB)]
    # eoT lo exists only as the fp8 word inside the SBUF-resident eoq8 in
    # the zero-bias program; the generic program still stages it in DRAM.
    eoT_l = [scratch(f"eoTl_scratch{i}") for i in range(NLB)]

    with tile.TileContext(nc) as tc:
        with (
            tc.tile_pool(name="const", bufs=1) as const,
            tc.tile_pool(name="exTb", bufs=1) as exTb_pool,
            tc.tile_pool(name="eoTb", bufs=2) as eoTb_pool,
            tc.tile_pool(name="psum_tp", bufs=2, space="PSUM") as psum_tp,
            tc.tile_pool(name="psum_mm", bufs=2, space="PSUM") as psum_mm,
            tc.tile_pool(name="psum_s4", bufs=4, space="PSUM") as psum_s4,
        ):
            identr = const.tile([P, P], F32R, name="identr")
            nc.gpsimd.dma_start(out=identr, in_=ident_in)
            identf = identr.bitcast(F32)
            identb = const.tile([P, P], BF16, name="identb")
            nc.scalar.copy(out=identb, in_=identr)

            if zero_bias:
                # SBUF-resident fp8 (Yl*2^13, Yh*2^4) pairs of the FULL
                # eoT, built once during the phase-A transposes.  Phase B's
                # align cross pass reads slices directly -- no per-l-block
                # eoT-lo streaming and no per-chunk yq rebuilds.
                eoq8_sb = const.tile([P, DG, 2, L], FP8, name="eoq8_sb")

            # b laid out [p, dg]: btile[p, dg] = b[dg*128 + p]
            # (only the generic-bias path reads it)
            if not zero_bias:
                btile = const.tile([P, DG], F32)
                nc.sync.dma_start(out=btile,
                                  in_=bvec.rearrange("(c p) -> p c", p=P))

            def transpose_128_group(src_row, dst, rdtype=False, bf=False):
                """Transpose four [128,128] slices through one PSUM bank;
                single eviction into dst ([128, 4, 128] SBUF AP).
                rdtype=False: fp32 transpose-mode (bit-exact, 2 cyc/row);
                rdtype=True: f32r mode (rounds to 11 bits, 1.5 cyc/row);
                bf=True: bf16 in/out (1 cyc/row)."""
                dt = BF16 if bf else (F32R if rdtype else F32)
                idt = identb if bf else (identr if rdtype else identf)
                tp = psum_tp.tile([P, 4 * P], dt, tag="tp", name="tpg")
                for i in range(4):
                    nc.tensor.transpose(
                        tp[:, i * P:(i + 1) * P],
                        src_row[:, i * P:(i + 1) * P],
                        idt,
                    )
                nc.scalar.copy(out=dst, in_=tp.rearrange(
                    "p (four c) -> p four c", four=4))

            _tp_rr = [0]

            def transpose_128_group_hl(src_row, dst_h, dst_l, borrow=False):
                """Like transpose_128_group, but evicts an f32r hi/lo pair:
                hi = rne11(psum) via ACT/DVE copy (alternating), lo = psum -
                hi via DVE sub.  borrow=True also rotates through the
                (phase-A-idle) stage-4 PSUM banks for a deeper transpose
                pipeline."""
                _tp_rr[0] += 1
                if borrow and _tp_rr[0] % 3 != 0:
                    tp = psum_s4.tile([P, 4 * P], F32, tag="s4",
                                      name=f"tpb{_tp_rr[0]}")
                else:
                    tp = psum_tp.tile([P, 4 * P], F32, tag="tp",
                                      name=f"tpt{_tp_rr[0]}")
                for i in range(4):
                    nc.tensor.transpose(
                        tp[:, i * P:(i + 1) * P],
                        src_row[:, i * P:(i + 1) * P],
                        identf,
                    )
                tp4 = tp.rearrange("p (four c) -> p four c", four=4)
                if _tp_rr[0] % 2 == 0:
                    nc.scalar.copy(out=dst_h, in_=tp4)
                else:
                    nc.vector.tensor_copy(out=dst_h, in_=tp4)
                nc.vector.tensor_sub(out=dst_l, in0=tp4, in1=dst_h)

            # ---------------- phase A: WTh/WTl + exT/eoT (hi/lo) -> DRAM ----
            with (
                tc.tile_pool(name="wt", bufs=1) as wt_pool,
                tc.tile_pool(name="stage", bufs=3) as stage_pool,
                tc.tile_pool(name="ev", bufs=1) as ev_pool,
                tc.tile_pool(name="evt", bufs=2) as evt_pool,
            ):
                # lhsT hi/lo pair for the ix projection:
                #   direct path: WT (transposed W);  G path: G = W^T@W
                #   (symmetric, so its [i-part, j] layout is its own lhsT)
                wth = wt_pool.tile([P, KC, D], F32R)
                if zero_bias:
                    # G-lo lives only as the fp8 word in wq8 (slot 1); no
                    # f32r wtl tile is kept in this mode.
                    wq8 = wt_pool.tile([P, KC, 2, D], FP8, name="wq8")
                else:
                    wtl = wt_pool.tile([P, KC, D], F32R)

                def tp_block(src_dram, dst_h, dst_l, pfx, blk, pool):
                    """dst = src_blk^T (hi/lo split), via small per-group
                    tiles DMA'd out immediately -- fills PE bubbles between
                    other phase-A work without big-slot contention.
                    zero_bias: hi also goes to DRAM (SWDGE queue -- the
                    sync queue carries W/ix loads), lo goes ONLY into the
                    resident eoq8 fp8 pairs."""
                    for s in range(4):
                        row = stage_pool.tile([P, D], F32, tag="stage",
                                              name=f"{pfx}row{blk}_{s}")
                        r0 = (blk * 4 + s) * P
                        (nc.gpsimd if zero_bias else nc.sync).dma_start(
                            out=row, in_=src_dram[r0:r0 + P, :])
                        ssl = slice(s * P, (s + 1) * P)
                        for q in range(2):
                            th = pool.tile([P, 4, P], F32R, tag="ioh",
                                           bufs=2,
                                           name=f"{pfx}h{blk}_{s}_{q}")
                            tl = pool.tile([P, 4, P], F32R, tag="iol",
                                           bufs=2,
                                           name=f"{pfx}l{blk}_{s}_{q}")
                            transpose_128_group_hl(
                                row[:, q * 4 * P:(q + 1) * 4 * P], th, tl)
                            qsl = slice(q * 4, (q + 1) * 4)
                            if zero_bias:
                                nc.gpsimd.dma_start(
                                    out=dst_h[blk][:, qsl, ssl], in_=th)
                                lr = slice(blk * 512 + s * P,
                                           blk * 512 + (s + 1) * P)
                                nc.scalar.activation(
                                    out=eoq8_sb[:, qsl, 1, lr], in_=th,
                                    func=COPYF, scale=16.0)
                                nc.vector.tensor_scalar_mul(
                                    out=eoq8_sb[:, qsl, 0, lr], in0=tl,
                                    scalar1=8192.0)
                            else:
                                nc.sync.dma_start(
                                    out=dst_h[blk][:, qsl, ssl], in_=th)
                                nc.sync.dma_start(
                                    out=dst_l[blk][:, qsl, ssl], in_=tl)

                if zero_bias:
                  # G = W^T@W via hi/lo.  G is symmetric, so the two cross
                  # terms are each other's transposes: Wh^T@Wl = (Wl^T@Wh)^T.
                  # C = Wh^T@Wl is a 2^-12-scale correction, so it runs in
                  # pure bf16 (err ~2^-21 * G).  C^T is also lo-scale, so it
                  # is added into the LO part after the hi/lo split, reading
                  # the transpose PSUM directly -- no C^T SBUF tensor.
                  with (
                      tc.tile_pool(name="split", bufs=2) as split_pool,
                      tc.tile_pool(name="cpool", bufs=1) as c_pool,
                  ):
                    if USE_C:
                        cmat = c_pool.tile([P, KC, D], BF16, name="cmat")
                        # fp8 caches of Wh*2^4 / Wl*2^16, laid out as
                        # DoubleRow pairs over adjacent 128-row contraction
                        # chunks: the C pass runs at 0.5 cyc/row.  C is a
                        # 2^-12-scale correction, so fp8's 2^-4 word
                        # precision gives C to 2^-16 overall.  Product
                        # scale 2^20 is removed at the cmat eviction.
                        whc8 = c_pool.tile([P, KC // 2, 2, D], FP8,
                                           name="whc8")
                        wlc8 = c_pool.tile([P, KC // 2, 2, D], FP8,
                                           name="wlc8")

                    def g_psums(pfx):
                        return ([psum_mm.tile([P, 512], F32, tag="mm",
                                              name=f"{pfx}_{i}")
                                 for i in range(2)]
                                + [psum_s4.tile([P, 512], F32, tag="s4",
                                                name=f"{pfx}_{i + 2}")
                                   for i in range(4)]
                                + [psum_tp.tile([P, 512], F32, tag="tp",
                                                name=f"{pfx}_{i + 6}")
                                   for i in range(2)])

                    # fp8 pair caches of Wh / Wl
                    for dc in range(DG if USE_C else 0):
                        wrow = stage_pool.tile([P, D], F32, tag="stage",
                                               name=f"gw{dc}")
                        nc.sync.dma_start(
                            out=wrow, in_=W[dc * P:(dc + 1) * P, :])
                        whr = split_pool.tile([P, D], F32R, tag="whc",
                                              name=f"gwh{dc}")
                        nc.vector.tensor_copy(out=whr, in_=wrow)
                        nc.scalar.activation(
                            out=whc8[:, dc // 2, dc % 2, :], in_=wrow,
                            func=COPYF, scale=16.0)
                        wl32 = split_pool.tile([P, D], F32, tag="wl32",
                                               name=f"gwl{dc}")
                        nc.vector.tensor_sub(out=wl32, in0=wrow, in1=whr)
                        nc.scalar.activation(
                            out=wlc8[:, dc // 2, dc % 2, :], in_=wl32,
                            func=COPYF, scale=65536.0)

                    # C = Wh^T @ Wl, fp8 DoubleRow (pairs = adjacent chunks)
                    for jh in range(2 if USE_C else 0):
                        jsl = slice(jh * 512, (jh + 1) * 512)
                        pss = g_psums(f"c{jh}")
                        for dp in range(DG // 2):
                            for ic in range(DG):
                                nc.tensor.matmul(
                                    pss[ic],
                                    whc8[:, dp, :, ic * P:(ic + 1) * P],
                                    wlc8[:, dp, :, jsl],
                                    start=(dp == 0), stop=(dp == DG // 2 - 1),
                                    perf_mode=DROW)
                        for ic in range(DG):
                            nc.scalar.activation(
                                out=cmat[:, ic, jsl], in_=pss[ic],
                                func=COPYF, scale=2.0 ** -20)

                    # G = Wh^T@Wh + C + C^T; Wh streamed per (jh, dc)
                    # (no-C mode computes G later, interleaved with the io
                    # transpose blocks)
                    for jh in range(2 if USE_C else 0):
                        jsl = slice(jh * 512, (jh + 1) * 512)
                        pss = g_psums(f"g{jh}")
                        for dc in range(DG):
                            # DMA W rows straight into an f32r tile (bit
                            # pattern is fp32; the PE read rounds to 11
                            # bits) -- no DVE conversion copy on the
                            # matmul critical path
                            whc = split_pool.tile([P, D], F32R, tag="whc",
                                                  name=f"g2wh{jh}_{dc}")
                            nc.sync.dma_start(
                                out=whc,
                                in_=W[dc * P:(dc + 1) * P, :].bitcast(F32R))
                            for ic in range(DG):
                                nc.tensor.matmul(
                                    pss[ic], whc[:, ic * P:(ic + 1) * P],
                                    whc[:, jsl],
                                    start=(dc == 0), stop=(dc == DG - 1))
                        for ic in range(DG):
                            if USE_C:
                                tmp = split_pool.tile(
                                    [P, 512], F32, tag="gtmp",
                                    name=f"ga{jh}_{ic}")
                                nc.vector.tensor_add(out=tmp, in0=pss[ic],
                                                     in1=cmat[:, ic, jsl])
                            else:
                                tmp = pss[ic]
                            nc.scalar.copy(out=wth[:, ic, jsl], in_=tmp)
                            gl = split_pool.tile([P, 512], F32, tag="gl",
                                                 name=f"gl{jh}_{ic}")
                            nc.vector.tensor_sub(out=gl, in0=tmp,
                                                 in1=wth[:, ic, jsl])
                            if USE_C:
                                # C^T via PE transposes of cmat, read from
                                # PSUM.  Allocated from psum_mm (not
                                # psum_tp): the tp slots are held by
                                # pss[6]/pss[7] until the last evictions,
                                # and the gl/gtmp slot chain feeds back
                                # into ctp -- psum_tp here would deadlock.
                                ctp = psum_mm.tile([P, 4 * P], BF16,
                                                   tag="mm",
                                                   name=f"ctp{jh}_{ic}")
                                for t in range(4):
                                    jc = jh * 4 + t
                                    nc.tensor.transpose(
                                        ctp[:, t * P:(t + 1) * P],
                                        cmat[:, jc, ic * P:(ic + 1) * P],
                                        identb)
                                nc.vector.tensor_add(
                                    out=gl, in0=gl,
                                    in1=ctp.rearrange(
                                        "p (four c) -> p four c", four=4))
                            # G-lo straight to its fp8 DoubleRow word
                            # (Gl*2^13); pairs with xh*2^-13 in the proj
                            # cross pass so both slot products land at
                            # scale 2^0 vs the f32r hi*hi pass and the
                            # cross pass accumulates into the SAME psum.
                            nc.scalar.activation(out=wq8[:, ic, 1, jsl],
                                                 in_=gl, func=COPYF,
                                                 scale=8192.0)
                    if USE_C:
                        # hi word: Gh*2^2, pairs with xl*2^-2
                        nc.scalar.activation(out=wq8[:, :, 0, :], in_=wth,
                                             func=COPYF, scale=4.0)
                    else:
                        # G = Wh^T@Wh in four 256-col quarter passes,
                        # ping-ponging across psum pools so each quarter's
                        # hi/lo eviction overlaps the next quarter's
                        # matmuls; io-transpose blocks are interleaved to
                        # keep the PE fed while evictions drain on ACT/DVE.
                        whcs = c_pool.tile([P, KC, D], F32R, name="whcs")
                        for dc in range(DG):
                            nc.sync.dma_start(
                                out=whcs[:, dc, :],
                                in_=W[dc * P:(dc + 1) * P, :].bitcast(F32R))
                        for jq in range(4):
                            jsl = slice(jq * 256, (jq + 1) * 256)
                            if jq % 2 == 0:
                                pss = ([psum_mm.tile(
                                    [P, 2, 256], F32, tag="mm",
                                    name=f"gq{jq}_{i}m") for i in range(2)]
                                    + [psum_s4.tile(
                                        [P, 2, 256], F32, tag="s4",
                                        name=f"gq{jq}_{i}s")
                                       for i in range(2)])
                            else:
                                pss = ([psum_s4.tile(
                                    [P, 2, 256], F32, tag="s4",
                                    name=f"gq{jq}_{i}s") for i in range(2)]
                                    + [psum_tp.tile(
                                        [P, 2, 256], F32, tag="tp",
                                        name=f"gq{jq}_{i}t")
                                       for i in range(2)])
                            # one accumulation group at a time per
                            # psum tile (two open groups in one zero
                            # region are rejected)
                            for ic in range(DG):
                                for dc in range(DG):
                                    nc.tensor.matmul(
                                        pss[ic // 2][:, ic % 2, :],
                                        whcs[:, dc, ic * P:(ic + 1) * P],
                                        whcs[:, dc, jsl],
                                        start=(dc == 0),
                                        stop=(dc == DG - 1))
                            for icp in range(4):
                                isl = slice(2 * icp, 2 * icp + 2)
                                nc.scalar.copy(out=wth[:, isl, jsl],
                                               in_=pss[icp])
                                gl = split_pool.tile(
                                    [P, 2, 256], F32, tag="gl",
                                    bufs=3, name=f"gl{jq}_{icp}")
                                nc.vector.tensor_sub(
                                    out=gl, in0=pss[icp],
                                    in1=wth[:, isl, jsl])
                                nc.scalar.activation(
                                    out=wq8[:, isl, 1, jsl],
                                    in_=gl, func=COPYF, scale=8192.0)
                            tp_block(iother, eoT_h, eoT_l, "eo", jq,
                                     split_pool)
                        nc.scalar.activation(out=wq8[:, :, 0, :],
                                             in_=wth, func=COPYF,
                                             scale=4.0)
                else:
                    for dc in range(DG):
                        wrow = stage_pool.tile([P, D], F32, tag="stage",
                                               name=f"wrow{dc}")
                        nc.sync.dma_start(out=wrow,
                                          in_=W[dc * P:(dc + 1) * P, :])
                        for q in range(2):
                            transpose_128_group_hl(
                                wrow[:, q * 4 * P:(q + 1) * 4 * P],
                                wth[:, q * 4:(q + 1) * 4, dc * P:(dc + 1) * P],
                                wtl[:, q * 4:(q + 1) * 4, dc * P:(dc + 1) * P],
                                borrow=True)

                with (
                    tc.tile_pool(name="xT", bufs=1) as xT_pool,
                    tc.tile_pool(name="ev", bufs=1) as ev_pool,
                    tc.tile_pool(name="evt", bufs=2) as evt_pool,
                    tc.tile_pool(name="iotp", bufs=2) as iotp_pool,
                ):
                    def proj_block(src_dram, dst_h, dst_l, pfx, blk):
                        """dst[blk] = lhsT_pair @ src_blk^T + b, stored
                        hi/lo.  zero_bias: x is transposed with a 4096*I
                        identity (x carries 2^12), the hi*hi pass runs in
                        f32r and the two cross terms run in ONE fp8
                        DoubleRow pass accumulating into the same psum.
                        Generic path: 3-pass f32r hi/lo.  Processed as two
                        256-halves with double-buffered xh/xl so half h+1's
                        transposes and evictions overlap half h's matmuls."""
                        for hf in range(2):
                            xh = xT_pool.tile([P, KC, 256], F32R, tag="xh",
                                              bufs=2, name=f"{pfx}xh{blk}_{hf}")
                            xl = xT_pool.tile([P, KC, 256], F32R, tag="xl",
                                              bufs=2, name=f"{pfx}xl{blk}_{hf}")
                            for si in range(2):
                                s_ = hf * 2 + si
                                row = stage_pool.tile(
                                    [P, D], F32, tag="stage",
                                    name=f"{pfx}row{blk}_{s_}")
                                r0 = (blk * 4 + s_) * P
                                nc.sync.dma_start(
                                    out=row, in_=src_dram[r0:r0 + P, :])
                                if zero_bias:
                                    # x carries 2^12 through proj and align
                                    # (see wq8 comment); exact 2^k scaling,
                                    # in place before the transpose
                                    nc.scalar.activation(
                                        out=row, in_=row, func=COPYF,
                                        scale=4096.0)
                                ssl = slice(si * P, (si + 1) * P)
                                for q in range(2):
                                    transpose_128_group_hl(
                                        row[:, q * 4 * P:(q + 1) * 4 * P],
                                        xh[:, q * 4:(q + 1) * 4, ssl],
                                        xl[:, q * 4:(q + 1) * 4, ssl],
                                        borrow=True)
                            if zero_bias:
                                xq8 = xT_pool.tile(
                                    [P, KC, 2, 256], FP8, tag="xq8", bufs=2,
                                    name=f"{pfx}xq8{blk}_{hf}")
                                q8h = ev_pool.tile(
                                    [P, DG, 256], FP8, tag="q8h", bufs=2,
                                    name=f"{pfx}q8h{blk}_{hf}")
                                q8l = ev_pool.tile(
                                    [P, DG, 256], FP8, tag="q8l", bufs=2,
                                    name=f"{pfx}q8l{blk}_{hf}")
                                nc.vector.tensor_scalar_mul(
                                    out=xq8[:, :, 0, :], in0=xl, scalar1=0.25)
                                nc.scalar.activation(
                                    out=xq8[:, :, 1, :], in_=xh, func=COPYF,
                                    scale=2.0 ** -13)
                            hsl = slice(hf * 256, (hf + 1) * 256)
                            for dg in range(DG):
                                if dg % 2 == 0:
                                    evh = ev_pool.tile(
                                        [P, 2, 256], F32R, tag="evh", bufs=2,
                                        name=f"{pfx}evh{blk}_{hf}_{dg}")
                                    evl = ev_pool.tile(
                                        [P, 2, 256], F32R, tag="evl", bufs=2,
                                        name=f"{pfx}evl{blk}_{hf}_{dg}")
                                if zero_bias:
                                    # two dg share one [P, 2, 256] psum
                                    # bank so the hi/lo eviction runs as a
                                    # single [P, 512] op pair
                                    if dg % 2 == 0:
                                        ps2 = psum_mm.tile(
                                            [P, 2, 256], F32, tag="mm",
                                            name=f"{pfx}ps{blk}_{hf}_{dg}")
                                    ps = ps2[:, dg % 2, :]
                                    for kc in range(KC):
                                        nc.tensor.matmul(
                                            ps,
                                            wth[:, kc, dg * P:(dg + 1) * P],
                                            xh[:, kc, :],
                                            start=(kc == 0), stop=False,
                                        )
                                    for kc in range(KC):
                                        nc.tensor.matmul(
                                            ps,
                                            wq8[:, kc, :, dg * P:(dg + 1) * P],
                                            xq8[:, kc, :, :],
                                            start=False, stop=(kc == KC - 1),
                                            perf_mode=DROW,
                                        )
                                else:
                                    ps = psum_mm.tile(
                                        [P, 256], F32, tag="mm",
                                        name=f"{pfx}ps{blk}_{hf}_{dg}")
                                    n = 0
                                    for wt_t, x_t in ((wth, xh), (wth, xl),
                                                      (wtl, xh)):
                                        for kc in range(KC):
                                            nc.tensor.matmul(
                                                ps,
                                                wt_t[:, kc, dg * P:(dg + 1) * P],
                                                x_t[:, kc, :],
                                                start=(n == 0),
                                                stop=(n == 3 * KC - 1),
                                            )
                                            n += 1
                                if zero_bias:
                                    if dg % 2 == 1:
                                        nc.scalar.copy(out=evh, in_=ps2)
                                        nc.vector.tensor_sub(
                                            out=evl, in0=ps2, in1=evh)
                                        # fp8 align-cross words, encoded
                                        # here (phase B just loads them)
                                        nc.scalar.activation(
                                            out=q8h[:, dg - 1:dg + 1, :],
                                            in_=evh,
                                            func=COPYF, scale=2.0 ** -13)
                                        nc.vector.tensor_scalar_mul(
                                            out=q8l[:, dg - 1:dg + 1, :],
                                            in0=evl,
                                            scalar1=2.0 ** -4)
                                else:
                                    tmp = evt_pool.tile(
                                        [P, 256], F32, tag="evt",
                                        name=f"{pfx}tmp{blk}_{hf}_{dg}")
                                    nc.vector.tensor_scalar_add(
                                        out=tmp, in0=ps,
                                        scalar1=btile[:, dg:dg + 1])
                                    nc.vector.tensor_copy(
                                        out=evh[:, dg % 2, :], in_=tmp)
                                    nc.vector.tensor_sub(
                                        out=evl[:, dg % 2, :], in0=tmp,
                                        in1=evh[:, dg % 2, :])
                                if dg % 2 == 1:
                                    dsl = slice(dg - 1, dg + 1)
                                    nc.sync.dma_start(
                                        out=dst_h[blk][:, dsl, hsl], in_=evh)
                                    if not zero_bias:
                                        # zero_bias keeps lo only as the
                                        # fp8 words (exq8*_s)
                                        nc.sync.dma_start(
                                            out=dst_l[blk][:, dsl, hsl],
                                            in_=evl)
                            if zero_bias:
                                nc.gpsimd.dma_start(
                                    out=exq8h_s[blk][:, :, hsl], in_=q8h)
                                nc.gpsimd.dma_start(
                                    out=exq8l_s[blk][:, :, hsl], in_=q8l)

                    if zero_bias:
                        if USE_C:
                            for blk in range(NLB):
                                tp_block(iother, eoT_h, eoT_l, "eo", blk,
                                         iotp_pool)
                        for blk in range(NLB):
                            proj_block(ix, exT_h, exT_l, "ex", blk)
                    else:
                        for blk in range(NLB):
                            proj_block(ix, exT_h, exT_l, "ex", blk)
                        for blk in range(NLB):
                            proj_block(iother, eoT_h, eoT_l, "eo", blk)

            # ---------------- phase B: align + softmax + output -------------
            with (
                tc.tile_pool(name="epool", bufs=4) as e_pool,
                tc.tile_pool(name="q8", bufs=1) as q8_pool,
                tc.tile_pool(name="c32", bufs=4) as c32_pool,
                tc.tile_pool(name="etpool", bufs=4) as et_pool,
                tc.tile_pool(name="s4rhs", bufs=6) as s4rhs_pool,
                tc.tile_pool(name="outp", bufs=6) as out_pool,
                tc.tile_pool(name="small", bufs=10) as small_pool,
            ):
                for lb in range(NLB):
                    exbh = exTb_pool.tile([P, DG, 512], F32R, tag="exbh",
                                          name=f"exbh{lb}")
                    # SWDGE queue: lets these overtake phase-A writes still
                    # pending in the sync-engine HWDGE FIFO
                    nc.gpsimd.dma_start(out=exbh, in_=exT_h[lb])
                    if not zero_bias:
                        exbl = exTb_pool.tile([P, DG, 512], F32R, tag="exbl",
                                              name=f"exbl{lb}")
                        nc.gpsimd.dma_start(out=exbl, in_=exT_l[lb])

                    NMC = 2 * MC      # 8 chunks of 256
                    Es = [e_pool.tile([P, L], F32, tag="E",
                                      name=f"E{lb}_{i}") for i in range(LS)]
                    nms = {}
                    if zero_bias:
                        # fp8 interleaved (Xh*2^-13, Xl*2^-4) pairs of exT
                        # (X carries 2^12), encoded in phase A -- just load.
                        # Paired with y words (Yl*2^13, Yh*2^4) both slot
                        # products land at scale 2^0 vs the f32r hi*hi pass,
                        # so the cross pass accumulates into the same psum.
                        exq8 = q8_pool.tile([P, 2, DG, 512], FP8, tag="exq8",
                                            name=f"exq8{lb}")
                        nc.gpsimd.dma_start(out=exq8[:, 0, :, :],
                                            in_=exq8h_s[lb])
                        nc.gpsimd.dma_start(out=exq8[:, 1, :, :],
                                            in_=exq8l_s[lb])
                    for mc in range(NMC):
                        msl = slice(mc * 256, (mc + 1) * 256)
                        blk_i, half = mc // 2, mc % 2
                        hsl = slice(half * 256, (half + 1) * 256)
                        eobh = eoTb_pool.tile([P, DG, 256], F32R, tag="eobh",
                                              name=f"eobh{lb}_{mc}")
                        nc.gpsimd.dma_start(out=eobh,
                                            in_=eoT_h[blk_i][:, :, hsl])
                        if zero_bias:
                            # fp8 (Yl*2^13, Yh*2^4) pairs come straight from
                            # the resident eoq8_sb -- no lo stream, no build
                            yq = eoq8_sb[:, :, :, msl]
                        else:
                            eobl = eoTb_pool.tile([P, DG, 256], F32R,
                                                  tag="eobl",
                                                  name=f"eobl{lb}_{mc}")
                            nc.gpsimd.dma_start(out=eobl,
                                                in_=eoT_l[blk_i][:, :, hsl])
                        for ls in range(LS):
                            xsl = slice(ls * P, (ls + 1) * P)
                            if zero_bias:
                                # hi*hi f32r pass + cross terms Xh@Yl+Xl@Yh
                                # in ONE fp8 DoubleRow continuation of the
                                # SAME psum accumulation (scales match)
                                ps = psum_mm.tile([P, 256], F32, tag="mm",
                                                  name=f"al{lb}_{mc}_{ls}")
                                for dc in range(DG):
                                    nc.tensor.matmul(
                                        ps, exbh[:, dc, xsl], eobh[:, dc, :],
                                        start=(dc == 0), stop=False,
                                    )
                                for dc in range(DG):
                                    nc.tensor.matmul(
                                        ps,
                                        exq8[:, :, dc, xsl],
                                        yq[:, dc, :, :],
                                        start=False,
                                        stop=(dc == DG - 1),
                                        perf_mode=DROW,
                                    )
                                # E is fp32: store raw 2^12-scaled logits;
                                # the PSUM is freed after this single read
                                # (alternate engines to balance load)
                                if (mc + ls) % 2 == 0:
                                    nc.vector.tensor_copy(
                                        out=Es[ls][:, msl], in_=ps)
                                else:
                                    nc.scalar.copy(
                                        out=Es[ls][:, msl], in_=ps)
                                if mc == 3:
                                    # first-half row max, hidden under the
                                    # align of chunks 4-7
                                    nms[ls] = small_pool.tile(
                                        [P, 1], F32, tag="nm1",
                                        name=f"nm1_{lb}_{ls}")
                                    nc.vector.reduce_max(
                                        nms[ls], Es[ls][:, :1024], axis=AX,
                                        negate=True)
                                continue
                            ps = psum_mm.tile([P, 256], F32, tag="mm",
                                              name=f"al{lb}_{mc}_{ls}")
                            n = 0
                            for x_t, eo_t in ((exbh, eobh), (exbh, eobl),
                                              (exbl, eobh)):
                                for dc in range(DG):
                                    nc.tensor.matmul(
                                        ps,
                                        x_t[:, dc, ls * P:(ls + 1) * P],
                                        eo_t[:, dc, :],
                                        start=(n == 0), stop=(n == 3 * DG - 1),
                                    )
                                    n += 1
                            nc.vector.tensor_copy(out=Es[ls][:, msl],
                                                    in_=ps)

                    ets = []
                    rzs = []
                    for ls in range(LS):
                        E = Es[ls]
                        negM = small_pool.tile([P, 1], F32, tag="negM",
                                               name=f"nm{lb}_{ls}")
                        if zero_bias:
                            nc.vector.reduce_max(negM, E[:, 1024:], axis=AX,
                                                 negate=True)
                            # -max(a,b) = min(-a,-b)
                            nc.vector.tensor_tensor(
                                out=negM, in0=negM, in1=nms[ls],
                                op=mybir.AluOpType.min)
                        else:
                            nc.vector.reduce_max(negM, E, axis=AX,
                                                 negate=True)
                        zsum = small_pool.tile([P, 1], F32, tag="zsum",
                                               name=f"zs{lb}_{ls}")
                        if zero_bias:
                            # logits carry 2^12; EXP's scale knob removes it
                            # (bias must then be -max * 2^-12 as well).
                            # exp output goes to an f32r tile: 1.5
                            # cyc/row E-transposes instead of fp32's 2
                            # (bf16 at 1.0 is rejected by the NEFF
                            # compiler when mixed with the f32r stage-4
                            # rhs: "Mixing of 32-bit and non-32-bit
                            # Matmult inputs not supported").
                            negMs = small_pool.tile([P, 1], F32, tag="negMs",
                                                    name=f"nms{lb}_{ls}")
                            nc.vector.tensor_scalar_mul(
                                out=negMs, in0=negM, scalar1=2.0 ** -12)
                            E2 = e_pool.tile([P, L], F32R, tag="E2",
                                             bufs=2, name=f"E2_{lb}_{ls}")
                            nc.scalar.activation(
                                out=E2, in_=E, func=EXP, bias=negMs,
                                scale=2.0 ** -12, accum_out=zsum)
                        else:
                            nc.scalar.activation(
                                out=E, in_=E, func=EXP, bias=negM, scale=1.0,
                                accum_out=zsum)
                        rz = small_pool.tile([P, 1], F32, tag="rz",
                                             name=f"rz{lb}_{ls}")
                        nc.vector.reciprocal(rz, zsum)
                        rzs.append(rz)
                        # ET[p, m16, l] = E[l, m16*128 + p]
                        ET = et_pool.tile([P, M16, P], F32R,
                                          tag="ET", name=f"ET{lb}_{ls}")
                        for q in range(4):
                            transpose_128_group(
                                (E2 if zero_bias else E)[:, q * 4 * P:
                                                         (q + 1) * 4 * P],
                                ET[:, q * 4:(q + 1) * 4, :],
                                rdtype=zero_bias)
                        ets.append(ET)

                    # stage 4: out rows = (E @ iother) * rz.  One [P, 1024]
                    # rhs load per m16 feeds BOTH d-halves (half the DMA
                    # dispatches); 8 psum banks (s4 + borrowed mm/tp, all
                    # idle here) hold the full 512x1024 output block.
                    pss4 = (
                        [psum_s4.tile([P, 512], F32, tag="s4",
                                      name=f"s4_{lb}_{i}") for i in range(4)]
                        + [psum_mm.tile([P, 512], F32, tag="mm",
                                        name=f"s4m_{lb}_{i}") for i in range(2)]
                        + [psum_tp.tile([P, 512], F32, tag="tp",
                                        name=f"s4t_{lb}_{i}") for i in range(2)]
                    )
                    for m16 in range(M16):
                        rhs = s4rhs_pool.tile([P, D], F32R, tag="s4rhs",
                                              name=f"rhs{lb}_{m16}")
                        # feed the wave from BOTH DMA queues: one queue
                        # alone cannot keep up with the PE
                        eng = nc.sync if m16 % 2 == 0 else nc.gpsimd
                        eng.dma_start(
                            out=rhs,
                            in_=iother[m16 * P:(m16 + 1) * P, :].bitcast(F32R))
                        for dg in range(2):
                            for ls in range(LS):
                                nc.tensor.matmul(
                                    pss4[dg * 4 + ls],
                                    ets[ls][:, m16, :],
                                    rhs[:, dg * 512:(dg + 1) * 512],
                                    start=(m16 == 0), stop=(m16 == M16 - 1),
                                )
                    for dg in range(2):
                        for ls in range(LS):
                            ot = out_pool.tile([P, 512], F32, tag="ot",
                                               name=f"ot{lb}_{dg}_{ls}")
                            if ls % 2 == 0:
                                nc.vector.tensor_scalar_mul(
                                    out=ot, in0=pss4[dg * 4 + ls],
                                    scalar1=rzs[ls])
                            else:
                                nc.scalar.activation(
                                    out=ot, in_=pss4[dg * 4 + ls],
                                    func=mybir.ActivationFunctionType.Copy,
                                    scale=rzs[ls])
                            r0 = lb * 512 + ls * P
                            nc.sync.dma_start(
                                out=out[r0:r0 + P, dg * 512:(dg + 1) * 512],
                                in_=ot)

    nc.compile()
    return nc


_NC_CACHE = {}


def _get_nc(zero_bias):
    if zero_bias not in _NC_CACHE:
        _NC_CACHE[zero_bias] = build_program(zero_bias)
    return _NC_CACHE[zero_bias]


def kernel(ix, iother, W, b):
    """Full-input entry point: shards batch across 8 NeuronCores."""
    from concourse.bass_utils import run_bass_kernel_spmd

    ix = np.ascontiguousarray(np.asarray(ix, dtype=np.float32))
    iother = np.ascontiguousarray(np.asarray(iother, dtype=np.float32))
    W = np.ascontiguousarray(np.asarray(W, dtype=np.float32))
    b = np.ascontiguousarray(np.asarray(b, dtype=np.float32))

    nc = _get_nc(zero_bias=bool(np.all(b == 0.0)))
    core_ids = list(range(NB))
    ident = np.eye(P, dtype=np.float32)
    in_maps = [
        {"ix": ix[i], "iother": iother[i], "W": W, "b": b, "ident": ident}
        for i in range(NB)
    ]
    res = run_bass_kernel_spmd(nc, in_maps, core_ids)
    outs = [res.results[i]["out"] for i in range(NB)]
    return np.stack(outs, axis=0).astype(np.float32)

